# revision 1
# baseline (speedup 1.0000x reference)
"""JPEG compression roundtrip kernel for Trainium2 (8 NeuronCores, batch-parallel).

Self-contained: builds constants, shards batch 32 -> 8 cores x 4 images,
runs a Bass/Tile kernel per core, gathers full output.

Pipeline per image (512x512x3 f32 in [0,1)):
  S1  u8 = floor(255*x) via rne(255*x - 0.5)           [ACT + DVE]
  p1  (stationary=u8 chunks, moving=color-scaled DCT)  -> M1 = (A@{Y,Cb,Cr})^T
  p2  (stationary=DCT const, moving=M1)                -> coef' [fw, fh]
  q   deq = rne(coef*1/t)*t                            [DVE, DVE, GPSIMD]
  p3  (stationary=deq chunks, moving=IDCT const)       -> M3 [fh, w]
  p4  (stationary=IDCT+color consts, moving=M3)        -> R,G,B planes in PSUM
  S5  out = min(max(rne(v),0),255)/255, interleave     [DVE, GPSIMD, ACT]

The 4:2:0 chroma down/upsample is folded into the chroma DCT matrices
(E = D@P, V = 2E^T); the +-128 level shifts cancel exactly because the DC
quant step (2) divides the DC shift (1024).
"""
import numpy as np

from concourse import bacc, bass, mybir, tile
from concourse.bass_utils import run_bass_kernel_spmd

F = np.float32
C_RNE = float(np.float32(12582912.0))  # 1.5 * 2**23
N_CORES = 8
B_PER_CORE = 4
DT = mybir.dt.float32
DT_MM = mybir.dt.float32
DT_BF = mybir.dt.bfloat16
QUALITY = 95

_LUMA = np.array([
    [16, 11, 10, 16, 24, 40, 51, 61],
    [12, 12, 14, 19, 26, 58, 60, 55],
    [14, 13, 16, 24, 40, 57, 69, 56],
    [14, 17, 22, 29, 51, 87, 80, 62],
    [18, 22, 37, 56, 68, 109, 103, 77],
    [24, 35, 55, 64, 81, 104, 113, 92],
    [49, 64, 78, 87, 103, 121, 120, 101],
    [72, 92, 95, 98, 112, 100, 103, 99]], dtype=F)
_CHROMA = np.array([
    [17, 18, 24, 47, 99, 99, 99, 99],
    [18, 21, 26, 66, 99, 99, 99, 99],
    [24, 26, 56, 99, 99, 99, 99, 99],
    [47, 66, 99, 99, 99, 99, 99, 99],
    [99, 99, 99, 99, 99, 99, 99, 99],
    [99, 99, 99, 99, 99, 99, 99, 99],
    [99, 99, 99, 99, 99, 99, 99, 99],
    [99, 99, 99, 99, 99, 99, 99, 99]], dtype=F)


def _qtable(base, quality):
    scale = 5000.0 / quality if quality < 50 else 200.0 - 2.0 * quality
    return np.clip(np.floor((base * scale + 50.0) / 100.0), 1.0, 255.0).astype(F)


def build_consts():
    k = np.arange(8)
    D = np.sqrt(2.0 / 8.0) * np.cos((2 * k[None, :] + 1) * k[:, None] * np.pi / 16.0)
    D[0, :] /= np.sqrt(2.0)
    D = D.astype(F)
    P = np.zeros((8, 16), F)
    for i in range(8):
        P[i, 2 * i] = 0.5
        P[i, 2 * i + 1] = 0.5
    E = (D @ P).astype(F)
    V = (2.0 * E.T).astype(F)
    QL = _qtable(_LUMA, QUALITY)
    QC = _qtable(_CHROMA, QUALITY)
    I16 = np.eye(16, dtype=F)
    I8 = np.eye(8, dtype=F)
    cY = np.array([0.299, 0.587, 0.114], F)
    cCb = np.array([-0.168736, -0.331264, 0.5], F)
    cCr = np.array([0.5, -0.418688, -0.081312], F)

    c = {}
    mv_fy = np.kron(I16, D.T).astype(F)
    mv_fc = np.kron(I8, E.T).astype(F)
    import ml_dtypes
    for ch in range(3):
        mv = np.ascontiguousarray(np.concatenate(
            [cY[ch] * mv_fy, cCb[ch] * mv_fc, cCr[ch] * mv_fc], axis=1).astype(F))
        hi = mv.astype(ml_dtypes.bfloat16)
        lo = (mv - hi.astype(F)).astype(ml_dtypes.bfloat16)
        c[f"mvp1_{ch}_hi"] = hi
        c[f"mvp1_{ch}_lo"] = lo
    c["sp2y"] = mv_fy.copy()
    c["sp2c"] = np.ascontiguousarray(np.pad(mv_fc, ((0, 0), (0, 64))))
    c["mvp3y"] = np.kron(I16, D).astype(F)
    c["mvp3c"] = np.kron(I16, V.T).astype(F)
    def _bfsplit(name, m):
        hi = m.astype(ml_dtypes.bfloat16)
        c[name + "_hi"] = hi
        c[name + "_lo"] = (m - hi.astype(F)).astype(ml_dtypes.bfloat16)
    _bfsplit("sp4y", np.kron(I16, D).astype(F))
    sp4c = np.kron(I16, V).T.astype(F)  # [128 fhc, 256 h]
    wR_cr, wG_cb, wG_cr, wB_cb = 1.402, -0.344136, -0.714136, 1.772
    for h in range(2):
        sl = np.ascontiguousarray(sp4c[:, 128 * h:128 * (h + 1)])
        _bfsplit(f"sp4c_h{h}_rcr", (F(wR_cr) * sl).astype(F))
        _bfsplit(f"sp4c_h{h}_gcb", (F(wG_cb) * sl).astype(F))
        _bfsplit(f"sp4c_h{h}_gcr", (F(wG_cr) * sl).astype(F))
        _bfsplit(f"sp4c_h{h}_bcb", (F(wB_cb) * sl).astype(F))
    tY = np.empty((128, 512), F)
    pp, ff = np.meshgrid(np.arange(128), np.arange(512), indexing="ij")
    tY[:] = QL[ff % 8, pp % 8]
    tC = np.empty((128, 256), F)
    pp, ff = np.meshgrid(np.arange(128), np.arange(256), indexing="ij")
    tC[:] = QC[ff % 8, pp % 8]
    c["taby"] = tY.reshape(128, 4, 128).copy()
    c["rtaby"] = (1.0 / tY).astype(F).reshape(128, 4, 128).copy()
    c["tabc"] = tC.reshape(128, 2, 128).copy()
    c["rtabc"] = (1.0 / tC).astype(F).reshape(128, 2, 128).copy()
    return c


BF_CONSTS = {"mvp1_0_hi", "mvp1_1_hi", "mvp1_2_hi", "mvp1_0_lo", "mvp1_1_lo", "mvp1_2_lo", "sp4y_hi", "sp4c_h0_rcr_hi", "sp4c_h0_gcb_hi", "sp4c_h0_gcr_hi", "sp4c_h0_bcb_hi", "sp4c_h1_rcr_hi", "sp4c_h1_gcb_hi", "sp4c_h1_gcr_hi", "sp4c_h1_bcb_hi", "sp4y_lo", "sp4c_h0_rcr_lo", "sp4c_h0_gcb_lo", "sp4c_h0_gcr_lo", "sp4c_h0_bcb_lo", "sp4c_h1_rcr_lo", "sp4c_h1_gcb_lo", "sp4c_h1_gcr_lo", "sp4c_h1_bcb_lo"}
MM_CONSTS = {"sp2y", "sp2c", "mvp3y", "mvp3c"}

CONST_SHAPES = {
    "mvp1_0_hi": (128, 256), "mvp1_0_lo": (128, 256),
    "mvp1_1_hi": (128, 256), "mvp1_1_lo": (128, 256),
    "mvp1_2_hi": (128, 256), "mvp1_2_lo": (128, 256),
    "sp2y": (128, 128), "sp2c": (128, 128),
    "mvp3y": (128, 128), "mvp3c": (128, 256),
    "sp4y_hi": (128, 128), "sp4y_lo": (128, 128),
    "sp4c_h0_rcr_hi": (128, 128), "sp4c_h0_rcr_lo": (128, 128),
    "sp4c_h0_gcb_hi": (128, 128), "sp4c_h0_gcb_lo": (128, 128),
    "sp4c_h0_gcr_hi": (128, 128), "sp4c_h0_gcr_lo": (128, 128),
    "sp4c_h0_bcb_hi": (128, 128), "sp4c_h0_bcb_lo": (128, 128),
    "sp4c_h1_rcr_hi": (128, 128), "sp4c_h1_rcr_lo": (128, 128),
    "sp4c_h1_gcb_hi": (128, 128), "sp4c_h1_gcb_lo": (128, 128),
    "sp4c_h1_gcr_hi": (128, 128), "sp4c_h1_gcr_lo": (128, 128),
    "sp4c_h1_bcb_hi": (128, 128), "sp4c_h1_bcb_lo": (128, 128),
    "taby": (128, 4, 128), "rtaby": (128, 4, 128),
    "tabc": (128, 2, 128), "rtabc": (128, 2, 128),
}


def _mm_ap(ap):
    return ap


def build_nc():
    Alu = mybir.AluOpType
    Act = mybir.ActivationFunctionType
    nc = bacc.Bacc("TRN2", target_bir_lowering=False, debug=False,
                   num_devices=N_CORES)
    x_d = nc.dram_tensor("x", [B_PER_CORE, 512, 512, 3], DT,
                         kind="ExternalInput").ap()
    o_d = nc.dram_tensor("out", [B_PER_CORE, 512, 512, 3], DT,
                         kind="ExternalOutput").ap()
    def _cdt(k):
        return DT_BF if k in BF_CONSTS else DT
    cd = {k: nc.dram_tensor(k, list(s), _cdt(k), kind="ExternalInput").ap()
          for k, s in CONST_SHAPES.items()}

    with tile.TileContext(nc) as tc:
        with (
            tc.tile_pool(name="cpool", bufs=1) as cpool,
            tc.tile_pool(name="iopool", bufs=3) as iopool,
            tc.tile_pool(name="u8pool", bufs=5) as u8pool,
            tc.tile_pool(name="m1pool", bufs=5) as m1pool,
            tc.tile_pool(name="m2pool", bufs=5) as m2pool,
            tc.tile_pool(name="m3pool", bufs=5) as m3pool,
            tc.tile_pool(name="ppool", bufs=4) as ppool,
            tc.tile_pool(name="pspool", bufs=6, space="PSUM") as pspool,
        ):
            ct = {}
            for k, s in CONST_SHAPES.items():
                ct[k] = cpool.tile(list(s), _cdt(k), tag=k, name=k)
                nc.sync.dma_start(out=ct[k][:], in_=cd[k][:])

            for b in range(B_PER_CORE):
                # ---- S1: load + floor(255*x) ----
                u8 = []
                for r in range(4):
                    xin = iopool.tile([128, 512, 3], DT, tag="xin", name="xin")
                    nc.sync.dma_start(out=xin[:], in_=x_d[b, 128 * r:128 * (r + 1)])
                    u8t = u8pool.tile([128, 512, 3], DT_BF, tag="u8", name="u8t")
                    nc.scalar.activation(xin[:], xin[:], Act.Copy,
                                         bias=-0.5, scale=255.0)
                    nc.vector.tensor_scalar(
                        out=u8t[:], in0=xin[:], scalar1=C_RNE, scalar2=C_RNE,
                        op0=Alu.add, op1=Alu.subtract)
                    u8.append(u8t)

                # ---- p1: M1 = (A @ plane)^T for Y/Cb/Cr at once ----
                m1y, m1cb, m1cr = [], [], []
                for jc in range(4):
                    psA = pspool.tile([128, 2, 256], DT, tag="ps", name="psA")
                    psB = pspool.tile([128, 2, 256], DT, tag="ps", name="psB")
                    for r in range(4):
                        pst = psA if r < 2 else psB
                        g = r % 2
                        idx = 0
                        for ch in range(3):
                            stat = u8[r][:, 128 * jc:128 * (jc + 1), ch]
                            for part in ("hi", "lo"):
                                nc.tensor.matmul(
                                    pst[:, g, :], stat,
                                    ct[f"mvp1_{ch}_{part}"][:],
                                    start=(idx == 0), stop=(idx == 5))
                                idx += 1
                    yt = m1pool.tile([128, 4, 128], DT_MM, tag="m1y", name="yt")
                    cbt = m1pool.tile([128, 4, 64], DT_MM, tag="m1cb", name="cbt")
                    crt = m1pool.tile([128, 4, 64], DT_MM, tag="m1cr", name="crt")
                    nc.scalar.copy(yt[:, 0:2, :], psA[:, :, 0:128])
                    nc.scalar.copy(yt[:, 2:4, :], psB[:, :, 0:128])
                    nc.vector.tensor_copy(cbt[:, 0:2, :], psA[:, :, 128:192])
                    nc.vector.tensor_copy(cbt[:, 2:4, :], psB[:, :, 128:192])
                    nc.vector.tensor_copy(crt[:, 0:2, :], psA[:, :, 192:256])
                    nc.vector.tensor_copy(crt[:, 2:4, :], psB[:, :, 192:256])
                    m1y.append(yt)
                    m1cb.append(cbt)
                    m1cr.append(crt)

                # ---- p2 + quant: luma ----
                m2qy = []
                for r2 in range(4):
                    ps2 = pspool.tile([128, 4, 128], DT, tag="ps", name="ps2")
                    nc.tensor.matmul(ps2[:], _mm_ap(ct["sp2y"][:]),
                                     _mm_ap(m1y[r2][:]), start=True, stop=True)
                    qt = m2pool.tile([128, 4, 128], DT_MM, tag="m2qy", name="qty")
                    nc.vector.tensor_tensor(
                        out=qt[:], in0=ps2[:], in1=ct["rtaby"][:], op=Alu.mult)
                    nc.vector.tensor_scalar(
                        out=qt[:], in0=qt[:], scalar1=C_RNE, scalar2=C_RNE,
                        op0=Alu.add, op1=Alu.subtract)
                    nc.gpsimd.tensor_tensor(
                        out=qt[:], in0=qt[:], in1=ct["taby"][:], op=Alu.mult)
                    m2qy.append(qt)

                # ---- p2 + quant: chroma (pairs of 64-row outputs) ----
                m2qc = {0: [], 1: []}
                for chi, m1c in ((0, m1cb), (1, m1cr)):
                    for t_ in range(2):
                        qt = m2pool.tile([128, 2, 128], DT_MM, tag="m2qc", name="qtc")
                        for half in range(2):
                            r2 = 2 * t_ + half
                            psc = pspool.tile([128, 2, 128], DT, tag="ps",
                                              name="psc")
                            nc.tensor.matmul(
                                psc[:], _mm_ap(ct["sp2c"][:]),
                                _mm_ap(m1c[r2][:]), start=True, stop=True)
                            nc.vector.tensor_tensor(
                                out=qt[64 * half:64 * (half + 1), :, :],
                                in0=psc[0:64, :, :], in1=ct["rtabc"][0:64, :, :],
                                op=Alu.mult)
                        nc.vector.tensor_scalar(
                            out=qt[:], in0=qt[:], scalar1=C_RNE, scalar2=C_RNE,
                            op0=Alu.add, op1=Alu.subtract)
                        nc.gpsimd.tensor_tensor(
                            out=qt[:], in0=qt[:], in1=ct["tabc"][:], op=Alu.mult)
                        m2qc[chi].append(qt)

                # ---- p3: luma -> M3 [fh, w] ----
                m3y = []
                for jc3 in range(4):
                    ps3 = pspool.tile([128, 4, 128], DT, tag="ps", name="ps3")
                    for r3 in range(4):
                        nc.tensor.matmul(
                            ps3[:, r3, :], _mm_ap(m2qy[r3][:, jc3, :]),
                            _mm_ap(ct["mvp3y"][:]), start=True, stop=True)
                    mt = m3pool.tile([128, 4, 128], DT_BF, tag="m3y", name="mty")
                    mtl = m3pool.tile([128, 4, 128], DT_BF, tag="m3yl", name="mtyl")
                    nc.scalar.copy(mt[:], ps3[:])
                    nc.vector.tensor_tensor(out=mtl[:], in0=ps3[:], in1=mt[:],
                                            op=Alu.subtract)
                    m3y.append((mt, mtl))

                # ---- p3: chroma -> M3c [fhc, w] ----
                m3c = {0: [], 1: []}
                for chi in (0, 1):
                    for jc3 in range(2):
                        ps3 = pspool.tile([128, 2, 256], DT, tag="ps", name="psA")
                        for r3 in range(2):
                            nc.tensor.matmul(
                                ps3[:, r3, :], _mm_ap(m2qc[chi][r3][:, jc3, :]),
                                _mm_ap(ct["mvp3c"][:]), start=True, stop=True)
                        mt = m3pool.tile([128, 2, 256], DT_BF, tag="m3c", name="mtc")
                        mtl = m3pool.tile([128, 2, 256], DT_BF, tag="m3cl", name="mtcl")
                        nc.scalar.copy(mt[:], ps3[:])
                        nc.vector.tensor_tensor(out=mtl[:], in0=ps3[:], in1=mt[:],
                                                op=Alu.subtract)
                        m3c[chi].append((mt, mtl))

                # ---- p4 + color + post + store ----
                for r in range(4):
                    rc, half = divmod(r, 2)
                    psR = pspool.tile([128, 512], DT, tag="ps", name="psR")
                    psG = pspool.tile([128, 512], DT, tag="ps", name="psG")
                    psB4 = pspool.tile([128, 512], DT, tag="ps", name="psB4")
                    my = m3y[r]
                    mcb = m3c[0][rc]
                    mcr = m3c[1][rc]

                    def _acc(ps, terms):
                        mms = []
                        for cname, (mh, ml) in terms:
                            sh = ct[cname + "_hi"][:]
                            sl = ct[cname + "_lo"][:]
                            mms += [(sh, mh[:]), (sh, ml[:]), (sl, mh[:])]
                        for i, (a_, b_) in enumerate(mms):
                            nc.tensor.matmul(ps[:], a_, b_, start=(i == 0),
                                             stop=(i == len(mms) - 1))
                    _acc(psR, [("sp4y", my), (f"sp4c_h{half}_rcr", mcr)])
                    _acc(psG, [("sp4y", my), (f"sp4c_h{half}_gcb", mcb),
                               (f"sp4c_h{half}_gcr", mcr)])
                    _acc(psB4, [("sp4y", my), (f"sp4c_h{half}_bcb", mcb)])
                    ot = iopool.tile([128, 512, 3], DT, tag="o", name="ot")
                    for chn, ps in ((0, psR), (1, psG), (2, psB4)):
                        pt = ppool.tile([128, 512], DT, tag="post", name="pt")
                        nc.vector.tensor_scalar(
                            out=pt[:], in0=ps[:], scalar1=C_RNE, scalar2=C_RNE,
                            op0=Alu.add, op1=Alu.subtract)
                        nc.gpsimd.tensor_scalar(
                            out=pt[:], in0=pt[:], scalar1=255.0, scalar2=0.0,
                            op0=Alu.min, op1=Alu.max)
                        nc.scalar.activation(ot[:, :, chn], pt[:], Act.Copy,
                                             bias=0.0, scale=float(F(1.0) / F(255.0)))
                    nc.sync.dma_start(out=o_d[b, 128 * r:128 * (r + 1)], in_=ot[:])

    nc.compile()
    return nc


_CACHE = {}


def kernel(x: np.ndarray) -> np.ndarray:
    assert x.shape == (32, 512, 512, 3)
    if "nc" not in _CACHE:
        _CACHE["nc"] = build_nc()
        _CACHE["consts"] = build_consts()
    nc = _CACHE["nc"]
    consts = _CACHE["consts"]
    xs = np.ascontiguousarray(x.astype(F))
    in_maps = []
    for i in range(N_CORES):
        m = {"x": xs[B_PER_CORE * i:B_PER_CORE * (i + 1)]}
        m.update(consts)
        in_maps.append(m)
    res = run_bass_kernel_spmd(nc, in_maps, list(range(N_CORES)))
    out = np.concatenate([res.results[i]["out"] for i in range(N_CORES)], axis=0)
    return out.astype(np.float32)



# revision 12
# speedup vs baseline: 2.0776x; 2.0776x over previous
"""JPEG compression roundtrip kernel for Trainium2 (8 NeuronCores, batch-parallel).

Self-contained: builds constants, shards batch 32 -> 8 cores x 4 images,
runs a Bass/Tile kernel per core, gathers full output.

v2 pipeline per image (512x512x3 f32 in [0,1)):
  S1  one op: u8' = fp16(255*x + 1023.5) = 1024 + floor(255*x)  (fp16 ulp=1
      in [1024,2048) does the rounding); engine rotates Act/DVE/Pool.
  p1  vertical DCT + color fold; the 1024 offset is cancelled by a 4th
      accumulated matmul (stationary=all-1024 const, moving=-colsums).
  p2  horizontal DCT with 1/t folded into 8 kv-split stationaries.
  q   rne via +/-C tensor_scalar (DVE) -> fp16; deq = *t fp16 tensor_tensor.
  p3  first inverse DCT dim (fp16).
  p4  second inverse + color with 1/255 folded; evict = clip[0,1] -> f32.

Level shifts cancel (DC quant step 2 divides 1024); final integer rounding
skipped (~2e-3 rel err, well within tolerance).
"""
import numpy as np

from concourse import bacc, bass, mybir, tile
from concourse.bass_utils import run_bass_kernel_spmd

F = np.float32
F16 = np.float16
C_RNE = float(np.float32(12582912.0))  # 1.5 * 2**23
C_U8 = 1024.0
N_CORES = 8
B_PER_CORE = 4
DT = mybir.dt.float32
DT16 = mybir.dt.float16
QUALITY = 95

_LUMA = np.array([
    [16, 11, 10, 16, 24, 40, 51, 61],
    [12, 12, 14, 19, 26, 58, 60, 55],
    [14, 13, 16, 24, 40, 57, 69, 56],
    [14, 17, 22, 29, 51, 87, 80, 62],
    [18, 22, 37, 56, 68, 109, 103, 77],
    [24, 35, 55, 64, 81, 104, 113, 92],
    [49, 64, 78, 87, 103, 121, 120, 101],
    [72, 92, 95, 98, 112, 100, 103, 99]], dtype=F)
_CHROMA = np.array([
    [17, 18, 24, 47, 99, 99, 99, 99],
    [18, 21, 26, 66, 99, 99, 99, 99],
    [24, 26, 56, 99, 99, 99, 99, 99],
    [47, 66, 99, 99, 99, 99, 99, 99],
    [99, 99, 99, 99, 99, 99, 99, 99],
    [99, 99, 99, 99, 99, 99, 99, 99],
    [99, 99, 99, 99, 99, 99, 99, 99],
    [99, 99, 99, 99, 99, 99, 99, 99]], dtype=F)


def _qtable(base, quality):
    scale = 5000.0 / quality if quality < 50 else 200.0 - 2.0 * quality
    return np.clip(np.floor((base * scale + 50.0) / 100.0), 1.0, 255.0).astype(F)


def build_consts():
    k = np.arange(8)
    D = np.sqrt(2.0 / 8.0) * np.cos((2 * k[None, :] + 1) * k[:, None] * np.pi / 16.0)
    D[0, :] /= np.sqrt(2.0)
    D = D.astype(F)
    P = np.zeros((8, 16), F)
    for i in range(8):
        P[i, 2 * i] = 0.5
        P[i, 2 * i + 1] = 0.5
    E = (D @ P).astype(F)          # [8,16]
    V = (2.0 * E.T).astype(F)      # [16,8]
    QL = _qtable(_LUMA, QUALITY)
    QC = _qtable(_CHROMA, QUALITY)
    I16 = np.eye(16, dtype=F)
    I8 = np.eye(8, dtype=F)
    cY = np.array([0.299, 0.587, 0.114], F)
    cCb = np.array([-0.168736, -0.331264, 0.5], F)
    cCr = np.array([0.5, -0.418688, -0.081312], F)

    mv_fy = np.kron(I16, D.T).astype(F)   # [128(g,i) , 128(g,kv)]
    mv_fc = np.kron(I8, E.T).astype(F)    # [128(vb,i), 64(vb,kv)]

    c = {}
    for ch in range(3):
        mv = np.concatenate(
            [cY[ch] * mv_fy, cCb[ch] * mv_fc, cCr[ch] * mv_fc], axis=1)
        c[f"mvp1_{ch}"] = np.ascontiguousarray(mv.astype(F16))  # [128,256]

    # offset compensation: cancel the +1024 carried by the u8' stationaries
    colsum = sum(c[f"mvp1_{ch}"].astype(np.float64).sum(axis=0)
                 for ch in range(3))
    mvcomp = np.zeros((128, 256), F16)
    mvcomp[0, :] = (-colsum).astype(F16)
    c["mvcomp"] = mvcomp
    c["c1024"] = np.full((128, 128), C_U8, F16)

    # p2 stationaries, kv-split with 1/t folded into columns (col = (cb,kh))
    khs = np.arange(128) % 8
    for kv in range(8):
        sy = mv_fy / QL[kv, khs][None, :]
        c[f"sp2y_{kv}"] = np.ascontiguousarray(sy.astype(F16))  # [128,128]
        sc = np.zeros((128, 128), F)
        sc[:, :64] = mv_fc / QC[kv, np.arange(64) % 8][None, :]
        c[f"sp2c_{kv}"] = np.ascontiguousarray(sc.astype(F16))  # [128,128]

    c["mvp3y"] = np.kron(I16, D).astype(F16)     # [128(cb,kh), 128(cb,j)]
    c["mvp3c"] = np.kron(I16, V.T).astype(F16)   # [128(hb,kh), 256(hb,j16)]

    c["sp4y"] = (np.kron(I16, D) / 255.0).astype(F16)  # [128(g,kv),128(g,i)]
    sp4c = np.kron(I16, V).T.astype(F)  # [128(vb16,kv8), 256(vb16,i16)]
    wR_cr, wG_cb, wG_cr, wB_cb = 1.402, -0.344136, -0.714136, 1.772
    for h in range(2):
        sl = sp4c[:, 128 * h:128 * (h + 1)] / 255.0
        c[f"sp4c_h{h}_rcr"] = np.ascontiguousarray((wR_cr * sl).astype(F16))
        c[f"sp4c_h{h}_gcb"] = np.ascontiguousarray((wG_cb * sl).astype(F16))
        c[f"sp4c_h{h}_gcr"] = np.ascontiguousarray((wG_cr * sl).astype(F16))
        c[f"sp4c_h{h}_bcb"] = np.ascontiguousarray((wB_cb * sl).astype(F16))

    # deq tables: qy layout [128 p=(cb,kh), 4 r, 16 g, 8 kv]: t = QL[kv, p%8]
    pp = np.arange(128) % 8
    taby = np.empty((128, 4, 16, 8), F)
    taby[:] = QL[np.arange(8)[None, None, None, :], pp[:, None, None, None]]
    c["taby"] = taby.astype(F16)
    # qc layout [128 p=(jcp,b,kh), 4 rv, 8 vb, 8 kv]: t = QC[kv, p%8]
    tabc = np.empty((128, 4, 8, 8), F)
    tabc[:] = QC[np.arange(8)[None, None, None, :], pp[:, None, None, None]]
    c["tabc"] = tabc.astype(F16)

    # pack all consts into one [128, CONST_W] fp16 array (single DMA)
    pak = np.zeros((128, CONST_W), F16)
    for kk, (off, w) in CONST_OFFS.items():
        pak[:, off:off + w] = c[kk].reshape(128, w)
    return {"cpak": pak}


CONST_SHAPES = {
    **{f"mvp1_{ch}": (128, 256) for ch in range(3)},
    "mvcomp": (128, 256), "c1024": (128, 128),
    **{f"sp2y_{kv}": (128, 128) for kv in range(8)},
    **{f"sp2c_{kv}": (128, 128) for kv in range(8)},
    "mvp3y": (128, 128), "mvp3c": (128, 256),
    "sp4y": (128, 128),
    **{f"sp4c_h{h}_{t}": (128, 128)
       for h in range(2) for t in ("rcr", "gcb", "gcr", "bcb")},
    "taby": (128, 4, 16, 8), "tabc": (128, 4, 8, 8),
}

CONST_OFFS = {}
_off = 0
for _k, _s in CONST_SHAPES.items():
    _w = int(np.prod(_s[1:]))
    CONST_OFFS[_k] = (_off, _w)
    _off += _w
CONST_W = _off


def build_nc():
    Alu = mybir.AluOpType
    Act = mybir.ActivationFunctionType
    nc = bacc.Bacc("TRN2", target_bir_lowering=False, debug=False,
                   num_devices=N_CORES)
    x_d = nc.dram_tensor("x", [B_PER_CORE, 512, 512, 3], DT,
                         kind="ExternalInput").ap()
    o_d = nc.dram_tensor("out", [B_PER_CORE, 512, 512, 3], DT,
                         kind="ExternalOutput").ap()
    cpak_d = nc.dram_tensor("cpak", [128, CONST_W], DT16,
                            kind="ExternalInput").ap()

    with tile.TileContext(nc) as tc:
        with (
            tc.tile_pool(name="cpool", bufs=1) as cpool,
            tc.tile_pool(name="iopool", bufs=10) as iopool,
            tc.tile_pool(name="u8pool", bufs=5) as u8pool,
            tc.tile_pool(name="m1pool", bufs=9) as m1pool,
            tc.tile_pool(name="qpool", bufs=9) as qpool,
            tc.tile_pool(name="m3pool", bufs=5) as m3pool,
            tc.tile_pool(name="ppool", bufs=4) as ppool,
            tc.tile_pool(name="otpool", bufs=6) as otpool,
            tc.tile_pool(name="pspool", bufs=8, space="PSUM") as pspool,
        ):
            cbig = cpool.tile([128, CONST_W], DT16, tag="cpak", name="cpak")
            crit = CONST_OFFS["c1024"][0] + CONST_OFFS["c1024"][1]
            nc.sync.dma_start(out=cbig[:, 0:crit], in_=cpak_d[:, 0:crit])
            nc.sync.dma_start(out=cbig[:, crit:], in_=cpak_d[:, crit:])
            ct = {k: cbig[:, off:off + w]
                  for k, (off, w) in CONST_OFFS.items()}

            st = {b: {} for b in range(B_PER_CORE)}

            def chunk_load(b):
                u8 = []
                for r in range(4):
                    xin = iopool.tile([128, 512, 3], DT, tag="xin", name="xin")
                    nc.sync.dma_start(out=xin[:], in_=x_d[b, 128 * r:128 * (r + 1)])
                    u8t = u8pool.tile([128, 512, 3], DT16, tag="u8", name="u8t")
                    if r % 2 == 0:
                        nc.scalar.activation(u8t[:], xin[:], Act.Copy,
                                             bias=C_U8 - 0.5, scale=255.0)
                    else:
                        eng = nc.vector if r == 1 else nc.gpsimd
                        eng.tensor_scalar(
                            out=u8t[:], in0=xin[:], scalar1=255.0,
                            scalar2=C_U8 - 0.5, op0=Alu.mult, op1=Alu.add)
                    u8.append(u8t)
                st[b]["u8"] = u8
                st[b]["m1"] = {}
                st[b]["qy"] = {}
                st[b]["pscc"] = {}
                st[b]["qc"] = {0: [None, None], 1: [None, None]}
                st[b]["m3y"] = {}
                st[b]["m3c"] = {0: {}, 1: {}}

            def chunk_p1(b, jc):
                # m1 tile layout: [128 col, 2 g, 32 grp, 8 kv]
                #   grp 0:16 = Y (g16), 16:24 = Cb (vb8), 24:32 = Cr (vb8)
                u8 = st[b]["u8"]
                pair = []
                for ab in range(2):
                    ps = pspool.tile([128, 2, 256], DT, tag="ps", name="psp1")
                    for g in range(2):
                        r = 2 * ab + g
                        for ch in range(3):
                            nc.tensor.matmul(
                                ps[:, g, :],
                                u8[r][:, 128 * jc:128 * (jc + 1), ch],
                                ct[f"mvp1_{ch}"],
                                start=(ch == 0), stop=False)
                        nc.tensor.matmul(
                            ps[:, g, :], ct["c1024"], ct["mvcomp"],
                            start=False, stop=True)
                    mt = m1pool.tile([128, 2, 32, 8], DT16, tag="m1",
                                     name="m1t")
                    if ab == 0:
                        nc.scalar.copy(mt[:], ps[:])
                    else:
                        nc.vector.tensor_copy(mt[:], ps[:])
                    pair.append(mt)
                st[b]["m1"][jc] = pair

            def chunk_p2y(b, jc):
                m1 = st[b]["m1"][jc]
                ps2 = pspool.tile([128, 4, 16, 8], DT, tag="ps", name="ps2")
                idx = 0
                for kv in range(8):
                    for ab in range(2):
                        nc.tensor.matmul(
                            ps2[:, 2 * ab:2 * ab + 2, :, kv],
                            ct[f"sp2y_{kv}"],
                            m1[ab][:, :, 0:16, kv],
                            start=(idx == 0), stop=(idx == 15))
                        idx += 1
                qt = qpool.tile([128, 4, 16, 8], DT16, tag="qy", name="qty")
                nc.vector.tensor_scalar(
                    out=qt[:], in0=ps2[:], scalar1=C_RNE, scalar2=C_RNE,
                    op0=Alu.add, op1=Alu.subtract)
                nc.vector.tensor_tensor(
                    out=qt[:], in0=qt[:], in1=ct["taby"], op=Alu.mult)
                st[b]["qy"][jc] = qt

            def chunk_p2c(b, jc):
                m1 = st[b]["m1"][jc]
                psc = pspool.tile([128, 2, 4, 8, 8], DT, tag="ps", name="psc")
                idx = 0
                for chi in range(2):
                    for kv in range(8):
                        for ab in range(2):
                            nc.tensor.matmul(
                                psc[:, chi, 2 * ab:2 * ab + 2, :, kv],
                                ct[f"sp2c_{kv}"],
                                m1[ab][:, :, 16 + 8 * chi:24 + 8 * chi, kv],
                                start=(idx == 0), stop=(idx == 31))
                            idx += 1
                st[b]["pscc"][jc] = psc

            def chunk_qc(b, t_):
                for chi in range(2):
                    qt = qpool.tile([128, 4, 8, 8], DT16, tag="qc", name="qtc")
                    for jcp in range(2):
                        psc = st[b]["pscc"][2 * t_ + jcp]
                        nc.vector.tensor_scalar(
                            out=qt[64 * jcp:64 * (jcp + 1)],
                            in0=psc[0:64, chi], scalar1=C_RNE,
                            scalar2=C_RNE, op0=Alu.add, op1=Alu.subtract)
                    nc.vector.tensor_tensor(
                        out=qt[:], in0=qt[:], in1=ct["tabc"], op=Alu.mult)
                    st[b]["qc"][chi][t_] = qt

            def chunk_p3y(b, r):
                qy = st[b]["qy"]
                ps3 = pspool.tile([128, 4, 128], DT, tag="ps", name="ps3")
                for jc in range(4):
                    nc.tensor.matmul(
                        ps3[:, jc, :], qy[jc][:, r], ct["mvp3y"],
                        start=(jc == 0), stop=(jc == 3))
                mt = m3pool.tile([128, 4, 128], DT16, tag="m3y", name="mty")
                nc.scalar.copy(mt[:], ps3[:])
                st[b]["m3y"][r] = mt

            def chunk_p3c(b, chi, h):
                qc = st[b]["qc"]
                ps3 = pspool.tile([128, 2, 256], DT, tag="ps", name="ps3c")
                for t_ in range(2):
                    nc.tensor.matmul(
                        ps3[:, t_, :], qc[chi][t_][:, 2 * h:2 * h + 2],
                        ct["mvp3c"],
                        start=(t_ == 0), stop=(t_ == 1))
                mt = m3pool.tile([128, 2, 256], DT16, tag="m3c", name="mtc")
                nc.scalar.copy(mt[:], ps3[:])
                st[b]["m3c"][chi][h] = mt

            def chunk_p4(b, r):
                rc, half = divmod(r, 2)
                psR = pspool.tile([128, 512], DT, tag="ps", name="psR")
                psG = pspool.tile([128, 512], DT, tag="ps", name="psG")
                psB4 = pspool.tile([128, 512], DT, tag="ps", name="psB4")
                my = st[b]["m3y"][r]
                mcb = st[b]["m3c"][0][rc]
                mcr = st[b]["m3c"][1][rc]
                nc.tensor.matmul(psR[:], ct["sp4y"], my[:],
                                 start=True, stop=False)
                nc.tensor.matmul(psG[:], ct["sp4y"], my[:],
                                 start=True, stop=False)
                nc.tensor.matmul(psB4[:], ct["sp4y"], my[:],
                                 start=True, stop=False)
                nc.tensor.matmul(psR[:], ct[f"sp4c_h{half}_rcr"],
                                 mcr[:], start=False, stop=True)
                nc.tensor.matmul(psG[:], ct[f"sp4c_h{half}_gcb"],
                                 mcb[:], start=False, stop=False)
                nc.tensor.matmul(psG[:], ct[f"sp4c_h{half}_gcr"],
                                 mcr[:], start=False, stop=True)
                nc.tensor.matmul(psB4[:], ct[f"sp4c_h{half}_bcb"],
                                 mcb[:], start=False, stop=True)
                ot = otpool.tile([128, 512, 3], DT, tag="o", name="ot")
                for chn, ps in ((0, psR), (2, psB4)):
                    nc.vector.tensor_scalar(
                        out=ot[:, :, chn], in0=ps[:], scalar1=1.0,
                        scalar2=0.0, op0=Alu.min, op1=Alu.max)
                pt = ppool.tile([128, 512], DT16, tag="pt", name="pt")
                nc.scalar.copy(pt[:], psG[:])
                nc.gpsimd.tensor_scalar(
                    out=ot[:, :, 1], in0=pt[:], scalar1=1.0,
                    scalar2=0.0, op0=Alu.min, op1=Alu.max)
                nc.gpsimd.dma_start(out=o_d[b, 128 * r:128 * (r + 1)],
                                    in_=ot[:])

            def step(b, p):
                # A-chunks for image b interleaved with B-chunks for image p
                has_a = b is not None
                has_p = p is not None
                if has_a:
                    chunk_load(b)
                    chunk_p1(b, 0)
                    chunk_p1(b, 1)
                if has_p:
                    chunk_p3c(p, 0, 0)
                    chunk_p3c(p, 1, 0)
                    chunk_p3y(p, 0)
                    chunk_p3y(p, 1)
                if has_a:
                    chunk_p1(b, 2)
                    chunk_p1(b, 3)
                if has_p:
                    chunk_p4(p, 0)
                    chunk_p4(p, 1)
                if has_a:
                    chunk_p2y(b, 0)
                    chunk_p2c(b, 0)
                    chunk_p2y(b, 1)
                    chunk_p2c(b, 1)
                    chunk_qc(b, 0)
                if has_p:
                    chunk_p3c(p, 0, 1)
                    chunk_p3c(p, 1, 1)
                    chunk_p3y(p, 2)
                    chunk_p3y(p, 3)
                if has_a:
                    chunk_p2y(b, 2)
                    chunk_p2c(b, 2)
                    chunk_p2y(b, 3)
                    chunk_p2c(b, 3)
                    chunk_qc(b, 1)
                if has_p:
                    chunk_p4(p, 2)
                    chunk_p4(p, 3)

            for s in range(B_PER_CORE + 1):
                step(s if s < B_PER_CORE else None,
                     s - 1 if s >= 1 else None)

    nc.compile()
    return nc


_CACHE = {}


def kernel(x: np.ndarray) -> np.ndarray:
    assert x.shape == (32, 512, 512, 3)
    if "nc" not in _CACHE:
        _CACHE["nc"] = build_nc()
        _CACHE["consts"] = build_consts()
    nc = _CACHE["nc"]
    consts = _CACHE["consts"]
    xs = np.ascontiguousarray(x.astype(F))
    in_maps = []
    for i in range(N_CORES):
        m = {"x": xs[B_PER_CORE * i:B_PER_CORE * (i + 1)]}
        m.update(consts)
        in_maps.append(m)
    res = run_bass_kernel_spmd(nc, in_maps, list(range(N_CORES)))
    out = np.concatenate([res.results[i]["out"] for i in range(N_CORES)], axis=0)
    return out.astype(np.float32)


# revision 18
# speedup vs baseline: 2.2057x; 1.0617x over previous
"""JPEG compression roundtrip kernel for Trainium2 (8 NeuronCores, batch-parallel).

Self-contained: builds constants, shards batch 32 -> 8 cores x 4 images,
runs a Bass/Tile kernel per core, gathers full output.

v2 pipeline per image (512x512x3 f32 in [0,1)):
  S1  one op: u8' = fp16(255*x + 1023.5) = 1024 + floor(255*x)  (fp16 ulp=1
      in [1024,2048) does the rounding); engine rotates Act/DVE/Pool.
  p1  vertical DCT + color fold; the 1024 offset is cancelled by a 4th
      accumulated matmul (stationary=all-1024 const, moving=-colsums).
  p2  horizontal DCT with 1/t folded into 8 kv-split stationaries.
  q   rne via +/-C tensor_scalar (DVE) -> fp16; deq = *t fp16 tensor_tensor.
  p3  first inverse DCT dim (fp16).
  p4  second inverse + color with 1/255 folded; evict = clip[0,1] -> f32.

Level shifts cancel (DC quant step 2 divides 1024); final integer rounding
skipped (~2e-3 rel err, well within tolerance).
"""
import numpy as np

from concourse import bacc, bass, mybir, tile
from concourse.bass_utils import run_bass_kernel_spmd

F = np.float32
F16 = np.float16
C_RNE = float(np.float32(12582912.0))  # 1.5 * 2**23
C_U8 = 1024.0
N_CORES = 8
B_PER_CORE = 4
DT = mybir.dt.float32
DT16 = mybir.dt.float16
QUALITY = 95

_LUMA = np.array([
    [16, 11, 10, 16, 24, 40, 51, 61],
    [12, 12, 14, 19, 26, 58, 60, 55],
    [14, 13, 16, 24, 40, 57, 69, 56],
    [14, 17, 22, 29, 51, 87, 80, 62],
    [18, 22, 37, 56, 68, 109, 103, 77],
    [24, 35, 55, 64, 81, 104, 113, 92],
    [49, 64, 78, 87, 103, 121, 120, 101],
    [72, 92, 95, 98, 112, 100, 103, 99]], dtype=F)
_CHROMA = np.array([
    [17, 18, 24, 47, 99, 99, 99, 99],
    [18, 21, 26, 66, 99, 99, 99, 99],
    [24, 26, 56, 99, 99, 99, 99, 99],
    [47, 66, 99, 99, 99, 99, 99, 99],
    [99, 99, 99, 99, 99, 99, 99, 99],
    [99, 99, 99, 99, 99, 99, 99, 99],
    [99, 99, 99, 99, 99, 99, 99, 99],
    [99, 99, 99, 99, 99, 99, 99, 99]], dtype=F)


def _qtable(base, quality):
    scale = 5000.0 / quality if quality < 50 else 200.0 - 2.0 * quality
    return np.clip(np.floor((base * scale + 50.0) / 100.0), 1.0, 255.0).astype(F)


def build_consts():
    k = np.arange(8)
    D = np.sqrt(2.0 / 8.0) * np.cos((2 * k[None, :] + 1) * k[:, None] * np.pi / 16.0)
    D[0, :] /= np.sqrt(2.0)
    D = D.astype(F)
    P = np.zeros((8, 16), F)
    for i in range(8):
        P[i, 2 * i] = 0.5
        P[i, 2 * i + 1] = 0.5
    E = (D @ P).astype(F)          # [8,16]
    V = (2.0 * E.T).astype(F)      # [16,8]
    QL = _qtable(_LUMA, QUALITY)
    QC = _qtable(_CHROMA, QUALITY)
    I16 = np.eye(16, dtype=F)
    I8 = np.eye(8, dtype=F)
    cY = np.array([0.299, 0.587, 0.114], F)
    cCb = np.array([-0.168736, -0.331264, 0.5], F)
    cCr = np.array([0.5, -0.418688, -0.081312], F)

    mv_fy = np.kron(I16, D.T).astype(F)   # [128(g,i) , 128(g,kv)]
    mv_fc = np.kron(I8, E.T).astype(F)    # [128(vb,i), 64(vb,kv)]

    c = {}
    for ch in range(3):
        mv = np.concatenate(
            [cY[ch] * mv_fy, cCb[ch] * mv_fc, cCr[ch] * mv_fc], axis=1)
        c[f"mvp1_{ch}"] = np.ascontiguousarray(mv.astype(F16))  # [128,256]

    # offset compensation: cancel the +1024 carried by the u8' stationaries
    colsum = sum(c[f"mvp1_{ch}"].astype(np.float64).sum(axis=0)
                 for ch in range(3))
    mvcomp = np.zeros((128, 256), F16)
    mvcomp[0, :] = (-colsum).astype(F16)
    c["mvcomp"] = mvcomp
    c["c1024"] = np.full((128, 128), C_U8, F16)

    # p2 stationaries, kv-split with 1/t folded into columns (col = (cb,kh))
    khs = np.arange(128) % 8
    for kv in range(8):
        sy = mv_fy / QL[kv, khs][None, :]
        c[f"sp2y_{kv}"] = np.ascontiguousarray(sy.astype(F16))  # [128,128]
        sc = np.zeros((128, 128), F)
        sc[:, :64] = mv_fc / QC[kv, np.arange(64) % 8][None, :]
        c[f"sp2c_{kv}"] = np.ascontiguousarray(sc.astype(F16))  # [128,128]

    c["mvp3y"] = np.kron(I16, D).astype(F16)     # [128(cb,kh), 128(cb,j)]
    c["mvp3c"] = np.kron(I16, V.T).astype(F16)   # [128(hb,kh), 256(hb,j16)]

    c["sp4y"] = np.kron(I16, D).astype(F16)  # [128(g,kv),128(g,i)]
    sp4c = np.kron(I16, V).T.astype(F)  # [128(vb16,kv8), 256(vb16,i16)]
    wR_cr, wG_cb, wG_cr, wB_cb = 1.402, -0.344136, -0.714136, 1.772
    for h in range(2):
        sl = sp4c[:, 128 * h:128 * (h + 1)]
        c[f"sp4c_h{h}_rcr"] = np.ascontiguousarray((wR_cr * sl).astype(F16))
        c[f"sp4c_h{h}_gcb"] = np.ascontiguousarray((wG_cb * sl).astype(F16))
        c[f"sp4c_h{h}_gcr"] = np.ascontiguousarray((wG_cr * sl).astype(F16))
        c[f"sp4c_h{h}_bcb"] = np.ascontiguousarray((wB_cb * sl).astype(F16))

    # deq tables: qy layout [128 p=(cb,kh), 4 r, 16 g, 8 kv]: t = QL[kv, p%8]
    pp = np.arange(128) % 8
    taby = np.empty((128, 4, 16, 8), F)
    taby[:] = QL[np.arange(8)[None, None, None, :], pp[:, None, None, None]]
    c["taby"] = taby.astype(F16)
    # qc layout [128 p=(jcp,b,kh), 4 rv, 8 vb, 8 kv]: t = QC[kv, p%8]
    tabc = np.empty((128, 4, 8, 8), F)
    tabc[:] = QC[np.arange(8)[None, None, None, :], pp[:, None, None, None]]
    c["tabc"] = tabc.astype(F16)

    # pack all consts into one [128, CONST_W] fp16 array (single DMA)
    pak = np.zeros((128, CONST_W), F16)
    for kk, (off, w) in CONST_OFFS.items():
        pak[:, off:off + w] = c[kk].reshape(128, w)
    return {"cpak": pak}


CONST_SHAPES = {
    **{f"mvp1_{ch}": (128, 256) for ch in range(3)},
    "mvcomp": (128, 256), "c1024": (128, 128),
    **{f"sp2y_{kv}": (128, 128) for kv in range(8)},
    **{f"sp2c_{kv}": (128, 128) for kv in range(8)},
    "mvp3y": (128, 128), "mvp3c": (128, 256),
    "sp4y": (128, 128),
    **{f"sp4c_h{h}_{t}": (128, 128)
       for h in range(2) for t in ("rcr", "gcb", "gcr", "bcb")},
    "taby": (128, 4, 16, 8), "tabc": (128, 4, 8, 8),
}

CONST_OFFS = {}
_off = 0
for _k, _s in CONST_SHAPES.items():
    _w = int(np.prod(_s[1:]))
    CONST_OFFS[_k] = (_off, _w)
    _off += _w
CONST_W = _off


def build_nc():
    Alu = mybir.AluOpType
    Act = mybir.ActivationFunctionType
    nc = bacc.Bacc("TRN2", target_bir_lowering=False, debug=False,
                   num_devices=N_CORES)
    x_d = nc.dram_tensor("x", [B_PER_CORE, 512, 512, 3], DT,
                         kind="ExternalInput").ap()
    o_d = nc.dram_tensor("out", [B_PER_CORE, 512, 512, 3], mybir.dt.uint8,
                         kind="ExternalOutput").ap()
    cpak_d = nc.dram_tensor("cpak", [128, CONST_W], DT16,
                            kind="ExternalInput").ap()

    with tile.TileContext(nc) as tc:
        with (
            tc.tile_pool(name="cpool", bufs=1) as cpool,
            tc.tile_pool(name="iopool", bufs=10) as iopool,
            tc.tile_pool(name="u8pool", bufs=9) as u8pool,
            tc.tile_pool(name="m1pool", bufs=9) as m1pool,
            tc.tile_pool(name="qpool", bufs=9) as qpool,
            tc.tile_pool(name="m3pool", bufs=5) as m3pool,
            tc.tile_pool(name="ppool", bufs=4) as ppool,
            tc.tile_pool(name="otpool", bufs=6) as otpool,
            tc.tile_pool(name="pspool", bufs=8, space="PSUM") as pspool,
        ):
            cbig = cpool.tile([128, CONST_W], DT16, tag="cpak", name="cpak")
            ct = {k: cbig[:, off:off + w]
                  for k, (off, w) in CONST_OFFS.items()}

            st = {b: {} for b in range(B_PER_CORE)}

            def chunk_load(b):
                u8 = []
                for r in range(4):
                    xin = iopool.tile([128, 512, 3], DT, tag="xin", name="xin")
                    nc.sync.dma_start(out=xin[:], in_=x_d[b, 128 * r:128 * (r + 1)])
                    u8t = u8pool.tile([128, 512, 3], DT16, tag="u8", name="u8t")
                    if r % 2 == 0:
                        nc.scalar.activation(u8t[:], xin[:], Act.Copy,
                                             bias=C_U8 - 0.5, scale=255.0)
                    else:
                        nc.gpsimd.tensor_scalar(
                            out=u8t[:], in0=xin[:], scalar1=255.0,
                            scalar2=C_U8 - 0.5, op0=Alu.mult, op1=Alu.add)
                    u8.append(u8t)
                st[b]["u8"] = u8
                st[b]["m1"] = {}
                st[b]["qy"] = {}
                st[b]["pscc"] = {}
                st[b]["qc"] = {0: [None, None], 1: [None, None]}
                st[b]["m3y"] = {}
                st[b]["m3c"] = {0: {}, 1: {}}

            def chunk_p1(b, jc):
                # m1 tile layout: [128 col, 2 g, 32 grp, 8 kv]
                #   grp 0:16 = Y (g16), 16:24 = Cb (vb8), 24:32 = Cr (vb8)
                u8 = st[b]["u8"]
                pair = []
                for ab in range(2):
                    ps = pspool.tile([128, 2, 256], DT, tag="ps", name="psp1")
                    for g in range(2):
                        r = 2 * ab + g
                        for ch in range(3):
                            nc.tensor.matmul(
                                ps[:, g, :],
                                u8[r][:, 128 * jc:128 * (jc + 1), ch],
                                ct[f"mvp1_{ch}"],
                                start=(ch == 0), stop=False)
                        nc.tensor.matmul(
                            ps[:, g, :], ct["c1024"], ct["mvcomp"],
                            start=False, stop=True)
                    mt = m1pool.tile([128, 2, 32, 8], DT16, tag="m1",
                                     name="m1t")
                    if ab == 0:
                        nc.scalar.copy(mt[:], ps[:])
                    else:
                        nc.vector.tensor_copy(mt[:], ps[:])
                    pair.append(mt)
                st[b]["m1"][jc] = pair

            def chunk_p2y(b, jc):
                m1 = st[b]["m1"][jc]
                ps2 = pspool.tile([128, 4, 16, 8], DT, tag="ps", name="ps2")
                idx = 0
                for kv in range(8):
                    for ab in range(2):
                        nc.tensor.matmul(
                            ps2[:, 2 * ab:2 * ab + 2, :, kv],
                            ct[f"sp2y_{kv}"],
                            m1[ab][:, :, 0:16, kv],
                            start=(idx == 0), stop=(idx == 15))
                        idx += 1
                qt = qpool.tile([128, 4, 16, 8], DT16, tag="qy", name="qty")
                nc.vector.tensor_scalar(
                    out=qt[:], in0=ps2[:], scalar1=C_RNE, scalar2=C_RNE,
                    op0=Alu.add, op1=Alu.subtract)
                nc.vector.tensor_tensor(
                    out=qt[:], in0=qt[:], in1=ct["taby"], op=Alu.mult)
                st[b]["qy"][jc] = qt

            def chunk_p2c(b, jc):
                m1 = st[b]["m1"][jc]
                psc = pspool.tile([128, 2, 4, 8, 8], DT, tag="ps", name="psc")
                idx = 0
                for chi in range(2):
                    for kv in range(8):
                        for ab in range(2):
                            nc.tensor.matmul(
                                psc[:, chi, 2 * ab:2 * ab + 2, :, kv],
                                ct[f"sp2c_{kv}"],
                                m1[ab][:, :, 16 + 8 * chi:24 + 8 * chi, kv],
                                start=(idx == 0), stop=(idx == 31))
                            idx += 1
                st[b]["pscc"][jc] = psc

            def chunk_qc(b, t_):
                for chi in range(2):
                    qt = qpool.tile([128, 4, 8, 8], DT16, tag="qc", name="qtc")
                    for jcp in range(2):
                        psc = st[b]["pscc"][2 * t_ + jcp]
                        nc.vector.tensor_scalar(
                            out=qt[64 * jcp:64 * (jcp + 1)],
                            in0=psc[0:64, chi], scalar1=C_RNE,
                            scalar2=C_RNE, op0=Alu.add, op1=Alu.subtract)
                    nc.vector.tensor_tensor(
                        out=qt[:], in0=qt[:], in1=ct["tabc"], op=Alu.mult)
                    st[b]["qc"][chi][t_] = qt

            def chunk_p3y(b, r):
                qy = st[b]["qy"]
                ps3 = pspool.tile([128, 4, 128], DT, tag="ps", name="ps3")
                for jc in range(4):
                    nc.tensor.matmul(
                        ps3[:, jc, :], qy[jc][:, r], ct["mvp3y"],
                        start=(jc == 0), stop=(jc == 3))
                mt = m3pool.tile([128, 4, 128], DT16, tag="m3y", name="mty")
                nc.scalar.copy(mt[:], ps3[:])
                st[b]["m3y"][r] = mt

            def chunk_p3c(b, chi, h):
                qc = st[b]["qc"]
                ps3 = pspool.tile([128, 2, 256], DT, tag="ps", name="ps3c")
                for t_ in range(2):
                    nc.tensor.matmul(
                        ps3[:, t_, :], qc[chi][t_][:, 2 * h:2 * h + 2],
                        ct["mvp3c"],
                        start=(t_ == 0), stop=(t_ == 1))
                mt = m3pool.tile([128, 2, 256], DT16, tag="m3c", name="mtc")
                nc.scalar.copy(mt[:], ps3[:])
                st[b]["m3c"][chi][h] = mt

            def chunk_p4(b, r):
                rc, half = divmod(r, 2)
                psR = pspool.tile([128, 512], DT, tag="ps", name="psR")
                psG = pspool.tile([128, 512], DT, tag="ps", name="psG")
                psB4 = pspool.tile([128, 512], DT, tag="ps", name="psB4")
                my = st[b]["m3y"][r]
                mcb = st[b]["m3c"][0][rc]
                mcr = st[b]["m3c"][1][rc]
                nc.tensor.matmul(psR[:], ct["sp4y"], my[:],
                                 start=True, stop=False)
                nc.tensor.matmul(psG[:], ct["sp4y"], my[:],
                                 start=True, stop=False)
                nc.tensor.matmul(psB4[:], ct["sp4y"], my[:],
                                 start=True, stop=False)
                nc.tensor.matmul(psR[:], ct[f"sp4c_h{half}_rcr"],
                                 mcr[:], start=False, stop=True)
                nc.tensor.matmul(psG[:], ct[f"sp4c_h{half}_gcb"],
                                 mcb[:], start=False, stop=False)
                nc.tensor.matmul(psG[:], ct[f"sp4c_h{half}_gcr"],
                                 mcr[:], start=False, stop=True)
                nc.tensor.matmul(psB4[:], ct[f"sp4c_h{half}_bcb"],
                                 mcb[:], start=False, stop=True)
                ot = otpool.tile([128, 512, 3], mybir.dt.uint8, tag="o",
                                 name="ot")
                for chn, ps in ((0, psR), (2, psB4)):
                    nc.vector.tensor_scalar(
                        out=ot[:, :, chn], in0=ps[:], scalar1=255.0,
                        scalar2=0.0, op0=Alu.min, op1=Alu.max)
                pt = ppool.tile([128, 512], DT16, tag="pt", name="pt")
                nc.scalar.copy(pt[:], psG[:])
                nc.gpsimd.tensor_scalar(
                    out=ot[:, :, 1], in0=pt[:], scalar1=255.0,
                    scalar2=0.0, op0=Alu.min, op1=Alu.max)
                nc.gpsimd.dma_start(out=o_d[b, 128 * r:128 * (r + 1)],
                                    in_=ot[:])

            def load_consts_rest():
                c1 = CONST_OFFS["c1024"][0] + CONST_OFFS["c1024"][1]
                c2 = CONST_OFFS["mvp3y"][0]
                tb = CONST_OFFS["taby"][0]
                nc.sync.dma_start(out=cbig[:, c1:c2], in_=cpak_d[:, c1:c2])
                nc.sync.dma_start(out=cbig[:, tb:], in_=cpak_d[:, tb:])
                nc.sync.dma_start(out=cbig[:, c2:tb], in_=cpak_d[:, c2:tb])

            def step(b, p):
                # A-chunks for image b interleaved with B-chunks for image p
                has_a = b is not None
                has_p = p is not None
                if b == 0:
                    c1 = CONST_OFFS["c1024"][0] + CONST_OFFS["c1024"][1]
                    nc.sync.dma_start(out=cbig[:, 0:c1], in_=cpak_d[:, 0:c1])
                    chunk_load(0)
                    load_consts_rest()
                if has_a:
                    chunk_p1(b, 0)
                    chunk_p1(b, 1)
                if has_p:
                    chunk_p3c(p, 0, 0)
                    chunk_p3c(p, 1, 0)
                    chunk_p3y(p, 0)
                    chunk_p3y(p, 1)
                if has_a:
                    chunk_p1(b, 2)
                    chunk_p1(b, 3)
                if has_p:
                    chunk_p4(p, 0)
                    chunk_p4(p, 1)
                if has_a and b + 1 < B_PER_CORE:
                    chunk_load(b + 1)
                if has_a:
                    chunk_p2y(b, 0)
                    chunk_p2c(b, 0)
                    chunk_p2y(b, 1)
                    chunk_p2c(b, 1)
                    chunk_qc(b, 0)
                if has_p:
                    chunk_p3c(p, 0, 1)
                    chunk_p3c(p, 1, 1)
                    chunk_p3y(p, 2)
                    chunk_p3y(p, 3)
                if has_a:
                    chunk_p2y(b, 2)
                    chunk_p2c(b, 2)
                    chunk_p2y(b, 3)
                    chunk_p2c(b, 3)
                    chunk_qc(b, 1)
                if has_p:
                    chunk_p4(p, 2)
                    chunk_p4(p, 3)

            for s in range(B_PER_CORE + 1):
                step(s if s < B_PER_CORE else None,
                     s - 1 if s >= 1 else None)

    nc.compile()
    return nc


_CACHE = {}


def kernel(x: np.ndarray) -> np.ndarray:
    assert x.shape == (32, 512, 512, 3)
    if "nc" not in _CACHE:
        _CACHE["nc"] = build_nc()
        _CACHE["consts"] = build_consts()
    nc = _CACHE["nc"]
    consts = _CACHE["consts"]
    xs = np.ascontiguousarray(x.astype(F))
    in_maps = []
    for i in range(N_CORES):
        m = {"x": xs[B_PER_CORE * i:B_PER_CORE * (i + 1)]}
        m.update(consts)
        in_maps.append(m)
    res = run_bass_kernel_spmd(nc, in_maps, list(range(N_CORES)))
    out = np.concatenate([res.results[i]["out"] for i in range(N_CORES)], axis=0)
    return out.astype(np.float32) / np.float32(255.0)


# revision 28
# speedup vs baseline: 2.3122x; 1.0483x over previous
"""JPEG compression roundtrip kernel for Trainium2 (8 NeuronCores, batch-parallel).

Self-contained: builds constants, shards batch 32 -> 8 cores x 4 images,
runs a Bass/Tile kernel per core, gathers full output.

v2 pipeline per image (512x512x3 f32 in [0,1)):
  S1  one op: u8' = fp16(255*x + 1023.5) = 1024 + floor(255*x)  (fp16 ulp=1
      in [1024,2048) does the rounding); engine rotates Act/DVE/Pool.
  p1  vertical DCT + color fold; the 1024 offset is cancelled by a 4th
      accumulated matmul (stationary=all-1024 const, moving=-colsums).
  p2  horizontal DCT with 1/t folded into 8 kv-split stationaries.
  q   rne via +/-C tensor_scalar (DVE) -> fp16; deq = *t fp16 tensor_tensor.
  p3  first inverse DCT dim (fp16).
  p4  second inverse + color with 1/255 folded; evict = clip[0,1] -> f32.

Level shifts cancel (DC quant step 2 divides 1024); final integer rounding
skipped (~2e-3 rel err, well within tolerance).
"""
import numpy as np

from concourse import bacc, bass, mybir, tile
from concourse.bass_utils import run_bass_kernel_spmd

F = np.float32
F16 = np.float16
C_RNE = float(np.float32(12582912.0))  # 1.5 * 2**23
C_U8 = 1024.0
N_CORES = 8
B_PER_CORE = 4
DT = mybir.dt.float32
DT16 = mybir.dt.float16
QUALITY = 95

_LUMA = np.array([
    [16, 11, 10, 16, 24, 40, 51, 61],
    [12, 12, 14, 19, 26, 58, 60, 55],
    [14, 13, 16, 24, 40, 57, 69, 56],
    [14, 17, 22, 29, 51, 87, 80, 62],
    [18, 22, 37, 56, 68, 109, 103, 77],
    [24, 35, 55, 64, 81, 104, 113, 92],
    [49, 64, 78, 87, 103, 121, 120, 101],
    [72, 92, 95, 98, 112, 100, 103, 99]], dtype=F)
_CHROMA = np.array([
    [17, 18, 24, 47, 99, 99, 99, 99],
    [18, 21, 26, 66, 99, 99, 99, 99],
    [24, 26, 56, 99, 99, 99, 99, 99],
    [47, 66, 99, 99, 99, 99, 99, 99],
    [99, 99, 99, 99, 99, 99, 99, 99],
    [99, 99, 99, 99, 99, 99, 99, 99],
    [99, 99, 99, 99, 99, 99, 99, 99],
    [99, 99, 99, 99, 99, 99, 99, 99]], dtype=F)


def _qtable(base, quality):
    scale = 5000.0 / quality if quality < 50 else 200.0 - 2.0 * quality
    return np.clip(np.floor((base * scale + 50.0) / 100.0), 1.0, 255.0).astype(F)


def build_consts():
    k = np.arange(8)
    D = np.sqrt(2.0 / 8.0) * np.cos((2 * k[None, :] + 1) * k[:, None] * np.pi / 16.0)
    D[0, :] /= np.sqrt(2.0)
    D = D.astype(F)
    P = np.zeros((8, 16), F)
    for i in range(8):
        P[i, 2 * i] = 0.5
        P[i, 2 * i + 1] = 0.5
    E = (D @ P).astype(F)          # [8,16]
    V = (2.0 * E.T).astype(F)      # [16,8]
    QL = _qtable(_LUMA, QUALITY)
    QC = _qtable(_CHROMA, QUALITY)
    I16 = np.eye(16, dtype=F)
    I8 = np.eye(8, dtype=F)
    cY = np.array([0.299, 0.587, 0.114], F)
    cCb = np.array([-0.168736, -0.331264, 0.5], F)
    cCr = np.array([0.5, -0.418688, -0.081312], F)

    mv_fy = np.kron(I16, D.T).astype(F)   # [128(g,i) , 128(g,kv)]
    mv_fc = np.kron(I8, E.T).astype(F)    # [128(vb,i), 64(vb,kv)]

    c = {}
    for ch in range(3):
        mv = np.concatenate(
            [cY[ch] * mv_fy, cCb[ch] * mv_fc, cCr[ch] * mv_fc], axis=1)
        c[f"mvp1_{ch}"] = np.ascontiguousarray(mv.astype(F16))  # [128,256]

    # offset compensation: cancel the +1024 carried by the u8' stationaries
    colsum = sum(c[f"mvp1_{ch}"].astype(np.float64).sum(axis=0)
                 for ch in range(3))
    mvcomp = np.zeros((128, 256), F16)
    mvcomp[0, :] = (-colsum).astype(F16)
    c["mvcomp"] = mvcomp
    c["c1024"] = np.full((128, 128), C_U8, F16)

    # p2 stationaries, kv-split with 1/t folded into columns (col = (cb,kh))
    khs = np.arange(128) % 8
    for kv in range(8):
        sy = mv_fy / QL[kv, khs][None, :]
        c[f"sp2y_{kv}"] = np.ascontiguousarray(sy.astype(F16))  # [128,128]
        sc = np.zeros((128, 128), F)
        sc[:, :64] = mv_fc / QC[kv, np.arange(64) % 8][None, :]
        c[f"sp2c_{kv}"] = np.ascontiguousarray(sc.astype(F16))  # [128,128]

    c["mvp3y"] = np.kron(I16, D).astype(F16)     # [128(cb,kh), 128(cb,j)]
    c["mvp3c"] = np.kron(I16, V.T).astype(F16)   # [128(hb,kh), 256(hb,j16)]

    c["sp4y"] = np.kron(I16, D).astype(F16)  # [128(g,kv),128(g,i)]
    sp4c = np.kron(I16, V).T.astype(F)  # [128(vb16,kv8), 256(vb16,i16)]
    wR_cr, wG_cb, wG_cr, wB_cb = 1.402, -0.344136, -0.714136, 1.772
    for h in range(2):
        sl = sp4c[:, 128 * h:128 * (h + 1)]
        c[f"sp4c_h{h}_rcr"] = np.ascontiguousarray((wR_cr * sl).astype(F16))
        c[f"sp4c_h{h}_gcb"] = np.ascontiguousarray((wG_cb * sl).astype(F16))
        c[f"sp4c_h{h}_gcr"] = np.ascontiguousarray((wG_cr * sl).astype(F16))
        c[f"sp4c_h{h}_bcb"] = np.ascontiguousarray((wB_cb * sl).astype(F16))

    # deq tables: qy layout [128 p=(cb,kh), 4 r, 16 g, 8 kv]: t = QL[kv, p%8]
    pp = np.arange(128) % 8
    taby = np.empty((128, 4, 16, 8), F)
    taby[:] = QL[np.arange(8)[None, None, None, :], pp[:, None, None, None]]
    c["taby"] = taby.astype(F16)
    # qc layout [128 p=(jcp,b,kh), 4 rv, 8 vb, 8 kv]: t = QC[kv, p%8]
    tabc = np.empty((128, 4, 8, 8), F)
    tabc[:] = QC[np.arange(8)[None, None, None, :], pp[:, None, None, None]]
    c["tabc"] = tabc.astype(F16)

    # pack all consts into one [128, CONST_W] fp16 array (single DMA)
    pak = np.zeros((128, CONST_W), F16)
    for kk, (off, w) in CONST_OFFS.items():
        pak[:, off:off + w] = c[kk].reshape(128, w)
    return {"cpak": pak}


CONST_SHAPES = {
    **{f"mvp1_{ch}": (128, 256) for ch in range(3)},
    "mvcomp": (128, 256), "c1024": (128, 128),
    **{f"sp2y_{kv}": (128, 128) for kv in range(8)},
    **{f"sp2c_{kv}": (128, 128) for kv in range(8)},
    "mvp3y": (128, 128), "mvp3c": (128, 256),
    "sp4y": (128, 128),
    **{f"sp4c_h{h}_{t}": (128, 128)
       for h in range(2) for t in ("rcr", "gcb", "gcr", "bcb")},
    "taby": (128, 4, 16, 8), "tabc": (128, 4, 8, 8),
}

CONST_OFFS = {}
_off = 0
for _k, _s in CONST_SHAPES.items():
    _w = int(np.prod(_s[1:]))
    CONST_OFFS[_k] = (_off, _w)
    _off += _w
CONST_W = _off


def build_nc():
    Alu = mybir.AluOpType
    Act = mybir.ActivationFunctionType
    nc = bacc.Bacc("TRN2", target_bir_lowering=False, debug=False,
                   num_devices=N_CORES)
    x_d = nc.dram_tensor("x", [B_PER_CORE, 512, 512, 3], DT,
                         kind="ExternalInput").ap()
    o_d = nc.dram_tensor("out", [B_PER_CORE, 512, 512, 3], mybir.dt.uint8,
                         kind="ExternalOutput").ap()
    cpak_d = nc.dram_tensor("cpak", [128, CONST_W], DT16,
                            kind="ExternalInput").ap()

    with tile.TileContext(nc) as tc:
        with (
            tc.tile_pool(name="cpool", bufs=1) as cpool,
            tc.tile_pool(name="iopool", bufs=10) as iopool,
            tc.tile_pool(name="u8pool", bufs=9) as u8pool,
            tc.tile_pool(name="m1pool", bufs=9) as m1pool,
            tc.tile_pool(name="qpool", bufs=9) as qpool,
            tc.tile_pool(name="m3pool", bufs=5) as m3pool,
            tc.tile_pool(name="ppool", bufs=4) as ppool,
            tc.tile_pool(name="otpool", bufs=6) as otpool,
            tc.tile_pool(name="pspool", bufs=8, space="PSUM") as pspool,
        ):
            cbig = cpool.tile([128, CONST_W], DT16, tag="cpak", name="cpak")
            ct = {k: cbig[:, off:off + w]
                  for k, (off, w) in CONST_OFFS.items()}

            st = {b: {} for b in range(B_PER_CORE)}

            def chunk_load_half(b, half):
                if half == 0:
                    st[b]["u8"] = [None] * 4
                    st[b]["m1"] = {}
                    st[b]["qy"] = {}
                    st[b]["pscc"] = {}
                    st[b]["qc"] = {0: [None, None], 1: [None, None]}
                    st[b]["m3y"] = {}
                    st[b]["m3c"] = {0: {}, 1: {}}
                for r in (0, 1) if half == 0 else (2, 3):
                    xin = iopool.tile([128, 512, 3], DT, tag="xin", name="xin")
                    nc.sync.dma_start(out=xin[:], in_=x_d[b, 128 * r:128 * (r + 1)])
                    u8t = u8pool.tile([128, 512, 3], DT16, tag="u8", name="u8t")
                    if r % 2 == 0:
                        nc.scalar.activation(u8t[:], xin[:], Act.Copy,
                                             bias=C_U8 - 0.5, scale=255.0)
                    else:
                        eng = nc.vector if b == 0 else nc.gpsimd
                        eng.tensor_scalar(
                            out=u8t[:], in0=xin[:], scalar1=255.0,
                            scalar2=C_U8 - 0.5, op0=Alu.mult, op1=Alu.add)
                    st[b]["u8"][r] = u8t

            def chunk_load(b):
                u8 = []
                for r in range(4):
                    xin = iopool.tile([128, 512, 3], DT, tag="xin", name="xin")
                    nc.sync.dma_start(out=xin[:], in_=x_d[b, 128 * r:128 * (r + 1)])
                    u8t = u8pool.tile([128, 512, 3], DT16, tag="u8", name="u8t")
                    if r % 2 == 0:
                        nc.scalar.activation(u8t[:], xin[:], Act.Copy,
                                             bias=C_U8 - 0.5, scale=255.0)
                    else:
                        eng = nc.vector if b == 0 else nc.gpsimd
                        eng.tensor_scalar(
                            out=u8t[:], in0=xin[:], scalar1=255.0,
                            scalar2=C_U8 - 0.5, op0=Alu.mult, op1=Alu.add)
                    u8.append(u8t)
                st[b]["u8"] = u8
                st[b]["m1"] = {}
                st[b]["qy"] = {}
                st[b]["pscc"] = {}
                st[b]["qc"] = {0: [None, None], 1: [None, None]}
                st[b]["m3y"] = {}
                st[b]["m3c"] = {0: {}, 1: {}}

            def chunk_p1(b, jc, ab):
                # m1 tile layout: [128 col, 4 r, 32 grp, 8 kv]
                #   grp 0:16 = Y (g16), 16:24 = Cb (vb8), 24:32 = Cr (vb8)
                u8 = st[b]["u8"]
                if ab == 0:
                    st[b]["m1"][jc] = m1pool.tile([128, 4, 32, 8], DT16,
                                                  tag="m1", name="m1t")
                mt = st[b]["m1"][jc]
                ps = pspool.tile([128, 2, 256], DT, tag="ps", name="psp1")
                for g in range(2):
                    r = 2 * ab + g
                    for ch in range(3):
                        nc.tensor.matmul(
                            ps[:, g, :],
                            u8[r][:, 128 * jc:128 * (jc + 1), ch],
                            ct[f"mvp1_{ch}"],
                            start=(ch == 0), stop=False)
                    nc.tensor.matmul(
                        ps[:, g, :], ct["c1024"], ct["mvcomp"],
                        start=False, stop=True)
                if ab == 0:
                    nc.scalar.copy(mt[:, 0:2], ps[:])
                else:
                    nc.vector.tensor_copy(mt[:, 2:4], ps[:])

            def chunk_p2y(b, jc):
                m1 = st[b]["m1"][jc]
                ps2 = pspool.tile([128, 4, 16, 8], DT, tag="ps", name="ps2")
                for kv in range(8):
                    nc.tensor.matmul(
                        ps2[:, :, :, kv], ct[f"sp2y_{kv}"],
                        m1[:, :, 0:16, kv],
                        start=(kv == 0), stop=(kv == 7))
                qt = qpool.tile([128, 4, 16, 8], DT16, tag="qy", name="qty")
                nc.vector.tensor_scalar(
                    out=qt[:], in0=ps2[:], scalar1=C_RNE, scalar2=C_RNE,
                    op0=Alu.add, op1=Alu.subtract)
                nc.vector.tensor_tensor(
                    out=qt[:], in0=qt[:], in1=ct["taby"], op=Alu.mult)
                st[b]["qy"][jc] = qt

            def chunk_p2c(b, jc):
                m1 = st[b]["m1"][jc]
                psc = pspool.tile([128, 2, 4, 8, 8], DT, tag="ps", name="psc")
                idx = 0
                for chi in range(2):
                    for kv in range(8):
                        nc.tensor.matmul(
                            psc[:, chi, :, :, kv], ct[f"sp2c_{kv}"],
                            m1[:, :, 16 + 8 * chi:24 + 8 * chi, kv],
                            start=(idx == 0), stop=(idx == 15))
                        idx += 1
                st[b]["pscc"][jc] = psc

            def chunk_qc(b, t_):
                for chi in range(2):
                    qt = qpool.tile([128, 4, 8, 8], DT16, tag="qc", name="qtc")
                    for jcp in range(2):
                        psc = st[b]["pscc"][2 * t_ + jcp]
                        nc.vector.tensor_scalar(
                            out=qt[64 * jcp:64 * (jcp + 1)],
                            in0=psc[0:64, chi], scalar1=C_RNE,
                            scalar2=C_RNE, op0=Alu.add, op1=Alu.subtract)
                    nc.vector.tensor_tensor(
                        out=qt[:], in0=qt[:], in1=ct["tabc"], op=Alu.mult)
                    st[b]["qc"][chi][t_] = qt

            def chunk_p3y(b, r):
                qy = st[b]["qy"]
                ps3 = pspool.tile([128, 4, 128], DT, tag="ps", name="ps3")
                for jc in range(4):
                    nc.tensor.matmul(
                        ps3[:, jc, :], qy[jc][:, r], ct["mvp3y"],
                        start=(jc == 0), stop=(jc == 3))
                mt = m3pool.tile([128, 4, 128], DT16, tag="m3y", name="mty")
                nc.scalar.copy(mt[:], ps3[:])
                st[b]["m3y"][r] = mt

            def chunk_p3c(b, chi, h):
                qc = st[b]["qc"]
                ps3 = pspool.tile([128, 2, 256], DT, tag="ps", name="ps3c")
                for t_ in range(2):
                    nc.tensor.matmul(
                        ps3[:, t_, :], qc[chi][t_][:, 2 * h:2 * h + 2],
                        ct["mvp3c"],
                        start=(t_ == 0), stop=(t_ == 1))
                mt = m3pool.tile([128, 2, 256], DT16, tag="m3c", name="mtc")
                nc.scalar.copy(mt[:], ps3[:])
                st[b]["m3c"][chi][h] = mt

            def chunk_p4(b, r):
                rc, half = divmod(r, 2)
                psR = pspool.tile([128, 512], DT, tag="ps", name="psR")
                psG = pspool.tile([128, 512], DT, tag="ps", name="psG")
                psB4 = pspool.tile([128, 512], DT, tag="ps", name="psB4")
                my = st[b]["m3y"][r]
                mcb = st[b]["m3c"][0][rc]
                mcr = st[b]["m3c"][1][rc]
                nc.tensor.matmul(psR[:], ct["sp4y"], my[:],
                                 start=True, stop=False)
                nc.tensor.matmul(psG[:], ct["sp4y"], my[:],
                                 start=True, stop=False)
                nc.tensor.matmul(psB4[:], ct["sp4y"], my[:],
                                 start=True, stop=False)
                nc.tensor.matmul(psR[:], ct[f"sp4c_h{half}_rcr"],
                                 mcr[:], start=False, stop=True)
                nc.tensor.matmul(psG[:], ct[f"sp4c_h{half}_gcb"],
                                 mcb[:], start=False, stop=False)
                nc.tensor.matmul(psG[:], ct[f"sp4c_h{half}_gcr"],
                                 mcr[:], start=False, stop=True)
                nc.tensor.matmul(psB4[:], ct[f"sp4c_h{half}_bcb"],
                                 mcb[:], start=False, stop=True)
                ot = otpool.tile([128, 512, 3], mybir.dt.uint8, tag="o",
                                 name="ot")
                last = (b == B_PER_CORE - 1 and r >= 2)
                for chn, ps in ((0, psR), (2, psB4)):
                    nc.vector.tensor_scalar(
                        out=ot[:, :, chn], in0=ps[:], scalar1=255.0,
                        scalar2=0.0, op0=Alu.min, op1=Alu.max)
                if last:
                    pt = ppool.tile([128, 512], DT16, tag="pt", name="pt")
                    nc.scalar.copy(pt[:], psG[:])
                    nc.gpsimd.tensor_scalar(
                        out=ot[:, :, 1], in0=pt[:], scalar1=255.0,
                        scalar2=0.0, op0=Alu.min, op1=Alu.max)
                    nc.sync.dma_start(out=o_d[b, 128 * r:128 * (r + 1)],
                                      in_=ot[:])
                else:
                    pt = ppool.tile([128, 512], DT16, tag="pt", name="pt")
                    nc.scalar.copy(pt[:], psG[:])
                    nc.gpsimd.tensor_scalar(
                        out=ot[:, :, 1], in0=pt[:], scalar1=255.0,
                        scalar2=0.0, op0=Alu.min, op1=Alu.max)
                    nc.gpsimd.dma_start(out=o_d[b, 128 * r:128 * (r + 1)],
                                        in_=ot[:])

            def load_consts_rest():
                c1 = CONST_OFFS["c1024"][0] + CONST_OFFS["c1024"][1]
                c2 = CONST_OFFS["mvp3y"][0]
                tb = CONST_OFFS["taby"][0]
                nc.sync.dma_start(out=cbig[:, c1:c2], in_=cpak_d[:, c1:c2])
                nc.sync.dma_start(out=cbig[:, tb:], in_=cpak_d[:, tb:])
                nc.sync.dma_start(out=cbig[:, c2:tb], in_=cpak_d[:, c2:tb])

            def step(b, p):
                # A-chunks for image b interleaved with B-chunks for image p
                has_a = b is not None
                has_p = p is not None
                if b == 0:
                    c1 = CONST_OFFS["c1024"][0] + CONST_OFFS["c1024"][1]
                    nc.sync.dma_start(out=cbig[:, 0:c1], in_=cpak_d[:, 0:c1])
                    chunk_load_half(0, 0)
                    chunk_load_half(0, 1)
                    chunk_load_half(1, 0)
                    load_consts_rest()
                if has_a:
                    for jc in range(4):
                        chunk_p1(b, jc, 0)
                if has_p:
                    chunk_p3c(p, 0, 0)
                    chunk_p3c(p, 1, 0)
                    chunk_p3y(p, 0)
                    chunk_p3y(p, 1)
                if b == 0:
                    chunk_load_half(1, 1)
                if has_a:
                    for jc in range(4):
                        chunk_p1(b, jc, 1)
                if has_p:
                    chunk_p4(p, 0)
                    chunk_p4(p, 1)
                if has_a and b + 1 < B_PER_CORE and b >= 1:
                    chunk_load_half(b + 1, 0)
                if has_a:
                    chunk_p2y(b, 0)
                    chunk_p2c(b, 0)
                    chunk_p2y(b, 1)
                    chunk_p2c(b, 1)
                    chunk_qc(b, 0)
                if has_p:
                    chunk_p3c(p, 0, 1)
                    chunk_p3c(p, 1, 1)
                    chunk_p3y(p, 2)
                    chunk_p3y(p, 3)
                if has_a and b + 1 < B_PER_CORE and b >= 1:
                    chunk_load_half(b + 1, 1)
                if has_a:
                    chunk_p2y(b, 2)
                    chunk_p2c(b, 2)
                    chunk_p2y(b, 3)
                    chunk_p2c(b, 3)
                    chunk_qc(b, 1)
                if has_p:
                    chunk_p4(p, 2)
                    chunk_p4(p, 3)

            for s in range(B_PER_CORE + 1):
                step(s if s < B_PER_CORE else None,
                     s - 1 if s >= 1 else None)

    nc.compile()
    return nc


_CACHE = {}


def kernel(x: np.ndarray) -> np.ndarray:
    assert x.shape == (32, 512, 512, 3)
    if "nc" not in _CACHE:
        _CACHE["nc"] = build_nc()
        _CACHE["consts"] = build_consts()
    nc = _CACHE["nc"]
    consts = _CACHE["consts"]
    xs = np.ascontiguousarray(x.astype(F))
    in_maps = []
    for i in range(N_CORES):
        m = {"x": xs[B_PER_CORE * i:B_PER_CORE * (i + 1)]}
        m.update(consts)
        in_maps.append(m)
    res = run_bass_kernel_spmd(nc, in_maps, list(range(N_CORES)))
    out = np.concatenate([res.results[i]["out"] for i in range(N_CORES)], axis=0)
    return out.astype(np.float32) / np.float32(255.0)


# revision 29
# speedup vs baseline: 2.3208x; 1.0037x over previous
"""JPEG compression roundtrip kernel for Trainium2 (8 NeuronCores, batch-parallel).

Self-contained: builds constants, shards batch 32 -> 8 cores x 4 images,
runs a Bass/Tile kernel per core, gathers full output.

v2 pipeline per image (512x512x3 f32 in [0,1)):
  S1  one op: u8' = fp16(255*x + 1023.5) = 1024 + floor(255*x)  (fp16 ulp=1
      in [1024,2048) does the rounding); engine rotates Act/DVE/Pool.
  p1  vertical DCT + color fold; the 1024 offset is cancelled by a 4th
      accumulated matmul (stationary=all-1024 const, moving=-colsums).
  p2  horizontal DCT with 1/t folded into 8 kv-split stationaries.
  q   rne via +/-C tensor_scalar (DVE) -> fp16; deq = *t fp16 tensor_tensor.
  p3  first inverse DCT dim (fp16).
  p4  second inverse + color with 1/255 folded; evict = clip[0,1] -> f32.

Level shifts cancel (DC quant step 2 divides 1024); final integer rounding
skipped (~2e-3 rel err, well within tolerance).
"""
import numpy as np

from concourse import bacc, bass, mybir, tile
from concourse.bass_utils import run_bass_kernel_spmd

F = np.float32
F16 = np.float16
C_RNE = float(np.float32(12582912.0))  # 1.5 * 2**23
C_U8 = 1024.0
N_CORES = 8
B_PER_CORE = 4
DT = mybir.dt.float32
DT16 = mybir.dt.float16
QUALITY = 95

_LUMA = np.array([
    [16, 11, 10, 16, 24, 40, 51, 61],
    [12, 12, 14, 19, 26, 58, 60, 55],
    [14, 13, 16, 24, 40, 57, 69, 56],
    [14, 17, 22, 29, 51, 87, 80, 62],
    [18, 22, 37, 56, 68, 109, 103, 77],
    [24, 35, 55, 64, 81, 104, 113, 92],
    [49, 64, 78, 87, 103, 121, 120, 101],
    [72, 92, 95, 98, 112, 100, 103, 99]], dtype=F)
_CHROMA = np.array([
    [17, 18, 24, 47, 99, 99, 99, 99],
    [18, 21, 26, 66, 99, 99, 99, 99],
    [24, 26, 56, 99, 99, 99, 99, 99],
    [47, 66, 99, 99, 99, 99, 99, 99],
    [99, 99, 99, 99, 99, 99, 99, 99],
    [99, 99, 99, 99, 99, 99, 99, 99],
    [99, 99, 99, 99, 99, 99, 99, 99],
    [99, 99, 99, 99, 99, 99, 99, 99]], dtype=F)


def _qtable(base, quality):
    scale = 5000.0 / quality if quality < 50 else 200.0 - 2.0 * quality
    return np.clip(np.floor((base * scale + 50.0) / 100.0), 1.0, 255.0).astype(F)


def build_consts():
    k = np.arange(8)
    D = np.sqrt(2.0 / 8.0) * np.cos((2 * k[None, :] + 1) * k[:, None] * np.pi / 16.0)
    D[0, :] /= np.sqrt(2.0)
    D = D.astype(F)
    P = np.zeros((8, 16), F)
    for i in range(8):
        P[i, 2 * i] = 0.5
        P[i, 2 * i + 1] = 0.5
    E = (D @ P).astype(F)          # [8,16]
    V = (2.0 * E.T).astype(F)      # [16,8]
    QL = _qtable(_LUMA, QUALITY)
    QC = _qtable(_CHROMA, QUALITY)
    I16 = np.eye(16, dtype=F)
    I8 = np.eye(8, dtype=F)
    cY = np.array([0.299, 0.587, 0.114], F)
    cCb = np.array([-0.168736, -0.331264, 0.5], F)
    cCr = np.array([0.5, -0.418688, -0.081312], F)

    mv_fy = np.kron(I16, D.T).astype(F)   # [128(g,i) , 128(g,kv)]
    mv_fc = np.kron(I8, E.T).astype(F)    # [128(vb,i), 64(vb,kv)]

    c = {}
    for ch in range(3):
        mv = np.concatenate(
            [cY[ch] * mv_fy, cCb[ch] * mv_fc, cCr[ch] * mv_fc], axis=1)
        c[f"mvp1_{ch}"] = np.ascontiguousarray(mv.astype(F16))  # [128,256]

    # offset compensation: cancel the +1024 carried by the u8' stationaries
    colsum = sum(c[f"mvp1_{ch}"].astype(np.float64).sum(axis=0)
                 for ch in range(3))
    mvcomp = np.zeros((128, 256), F16)
    mvcomp[0, :] = (-colsum).astype(F16)
    c["mvcomp"] = mvcomp
    c["c1024"] = np.full((128, 128), C_U8, F16)

    # p2 stationaries, kv-split with 1/t folded into columns (col = (cb,kh))
    khs = np.arange(128) % 8
    for kv in range(8):
        sy = mv_fy / QL[kv, khs][None, :]
        c[f"sp2y_{kv}"] = np.ascontiguousarray(sy.astype(F16))  # [128,128]
        sc = np.zeros((128, 128), F)
        sc[:, :64] = mv_fc / QC[kv, np.arange(64) % 8][None, :]
        c[f"sp2c_{kv}"] = np.ascontiguousarray(sc.astype(F16))  # [128,128]

    c["mvp3y"] = np.kron(I16, D).astype(F16)     # [128(cb,kh), 128(cb,j)]
    c["mvp3c"] = np.kron(I16, V.T).astype(F16)   # [128(hb,kh), 256(hb,j16)]

    c["sp4y"] = np.kron(I16, D).astype(F16)  # [128(g,kv),128(g,i)]
    sp4c = np.kron(I16, V).T.astype(F)  # [128(vb16,kv8), 256(vb16,i16)]
    wR_cr, wG_cb, wG_cr, wB_cb = 1.402, -0.344136, -0.714136, 1.772
    for h in range(2):
        sl = sp4c[:, 128 * h:128 * (h + 1)]
        c[f"sp4c_h{h}_rcr"] = np.ascontiguousarray((wR_cr * sl).astype(F16))
        c[f"sp4c_h{h}_gcb"] = np.ascontiguousarray((wG_cb * sl).astype(F16))
        c[f"sp4c_h{h}_gcr"] = np.ascontiguousarray((wG_cr * sl).astype(F16))
        c[f"sp4c_h{h}_bcb"] = np.ascontiguousarray((wB_cb * sl).astype(F16))

    # deq tables: qy layout [128 p=(cb,kh), 4 r, 16 g, 8 kv]: t = QL[kv, p%8]
    pp = np.arange(128) % 8
    taby = np.empty((128, 4, 16, 8), F)
    taby[:] = QL[np.arange(8)[None, None, None, :], pp[:, None, None, None]]
    c["taby"] = taby.astype(F16)
    # qc layout [128 p=(jcp,b,kh), 4 rv, 8 vb, 8 kv]: t = QC[kv, p%8]
    tabc = np.empty((128, 4, 8, 8), F)
    tabc[:] = QC[np.arange(8)[None, None, None, :], pp[:, None, None, None]]
    c["tabc"] = tabc.astype(F16)

    # pack all consts into one [128, CONST_W] fp16 array (single DMA)
    pak = np.zeros((128, CONST_W), F16)
    for kk, (off, w) in CONST_OFFS.items():
        pak[:, off:off + w] = c[kk].reshape(128, w)
    return {"cpak": pak}


CONST_SHAPES = {
    **{f"mvp1_{ch}": (128, 256) for ch in range(3)},
    "mvcomp": (128, 256), "c1024": (128, 128),
    **{f"sp2y_{kv}": (128, 128) for kv in range(8)},
    **{f"sp2c_{kv}": (128, 128) for kv in range(8)},
    "mvp3y": (128, 128), "mvp3c": (128, 256),
    "sp4y": (128, 128),
    **{f"sp4c_h{h}_{t}": (128, 128)
       for h in range(2) for t in ("rcr", "gcb", "gcr", "bcb")},
    "taby": (128, 4, 16, 8), "tabc": (128, 4, 8, 8),
}

CONST_OFFS = {}
_off = 0
for _k, _s in CONST_SHAPES.items():
    _w = int(np.prod(_s[1:]))
    CONST_OFFS[_k] = (_off, _w)
    _off += _w
CONST_W = _off


def build_nc():
    Alu = mybir.AluOpType
    Act = mybir.ActivationFunctionType
    nc = bacc.Bacc("TRN2", target_bir_lowering=False, debug=False,
                   num_devices=N_CORES)
    x_d = nc.dram_tensor("x", [B_PER_CORE, 512, 512, 3], DT,
                         kind="ExternalInput").ap()
    o_d = nc.dram_tensor("out", [B_PER_CORE, 512, 512, 3], mybir.dt.uint8,
                         kind="ExternalOutput").ap()
    cpak_d = nc.dram_tensor("cpak", [128, CONST_W], DT16,
                            kind="ExternalInput").ap()

    with tile.TileContext(nc) as tc:
        with (
            tc.tile_pool(name="cpool", bufs=1) as cpool,
            tc.tile_pool(name="iopool", bufs=10) as iopool,
            tc.tile_pool(name="u8pool", bufs=9) as u8pool,
            tc.tile_pool(name="m1pool", bufs=9) as m1pool,
            tc.tile_pool(name="qpool", bufs=9) as qpool,
            tc.tile_pool(name="m3pool", bufs=5) as m3pool,
            tc.tile_pool(name="ppool", bufs=4) as ppool,
            tc.tile_pool(name="otpool", bufs=6) as otpool,
            tc.tile_pool(name="pspool", bufs=8, space="PSUM") as pspool,
        ):
            cbig = cpool.tile([128, CONST_W], DT16, tag="cpak", name="cpak")
            ct = {k: cbig[:, off:off + w]
                  for k, (off, w) in CONST_OFFS.items()}

            st = {b: {} for b in range(B_PER_CORE)}

            def chunk_load_half(b, half):
                if half == 0:
                    st[b]["u8"] = [None] * 4
                    st[b]["m1"] = {}
                    st[b]["qy"] = {}
                    st[b]["pscc"] = {}
                    st[b]["qc"] = {0: [None, None], 1: [None, None]}
                    st[b]["m3y"] = {}
                    st[b]["m3c"] = {0: {}, 1: {}}
                if b == 0 and half == 0:
                    # column-split first loads so p1 can start on cols 0:256
                    xh = {}
                    for ch_ in range(2):
                        for r in (0, 1):
                            t = iopool.tile([128, 256, 3], DT, tag="xinh",
                                            name="xinh")
                            nc.sync.dma_start(
                                out=t[:],
                                in_=x_d[b, 128 * r:128 * (r + 1),
                                        256 * ch_:256 * (ch_ + 1)])
                            xh[(r, ch_)] = t
                    for r in (0, 1):
                        u8t = u8pool.tile([128, 512, 3], DT16, tag="u8",
                                          name="u8t")
                        for ch_ in range(2):
                            if r == 0:
                                nc.scalar.activation(
                                    u8t[:, 256 * ch_:256 * (ch_ + 1)],
                                    xh[(r, ch_)][:], Act.Copy,
                                    bias=C_U8 - 0.5, scale=255.0)
                            else:
                                nc.vector.tensor_scalar(
                                    out=u8t[:, 256 * ch_:256 * (ch_ + 1)],
                                    in0=xh[(r, ch_)][:], scalar1=255.0,
                                    scalar2=C_U8 - 0.5, op0=Alu.mult,
                                    op1=Alu.add)
                        st[b]["u8"][r] = u8t
                    return
                for r in (0, 1) if half == 0 else (2, 3):
                    xin = iopool.tile([128, 512, 3], DT, tag="xin", name="xin")
                    nc.sync.dma_start(out=xin[:], in_=x_d[b, 128 * r:128 * (r + 1)])
                    u8t = u8pool.tile([128, 512, 3], DT16, tag="u8", name="u8t")
                    if r % 2 == 0:
                        nc.scalar.activation(u8t[:], xin[:], Act.Copy,
                                             bias=C_U8 - 0.5, scale=255.0)
                    else:
                        eng = nc.vector if b == 0 else nc.gpsimd
                        eng.tensor_scalar(
                            out=u8t[:], in0=xin[:], scalar1=255.0,
                            scalar2=C_U8 - 0.5, op0=Alu.mult, op1=Alu.add)
                    st[b]["u8"][r] = u8t

            def chunk_load(b):
                u8 = []
                for r in range(4):
                    xin = iopool.tile([128, 512, 3], DT, tag="xin", name="xin")
                    nc.sync.dma_start(out=xin[:], in_=x_d[b, 128 * r:128 * (r + 1)])
                    u8t = u8pool.tile([128, 512, 3], DT16, tag="u8", name="u8t")
                    if r % 2 == 0:
                        nc.scalar.activation(u8t[:], xin[:], Act.Copy,
                                             bias=C_U8 - 0.5, scale=255.0)
                    else:
                        eng = nc.vector if b == 0 else nc.gpsimd
                        eng.tensor_scalar(
                            out=u8t[:], in0=xin[:], scalar1=255.0,
                            scalar2=C_U8 - 0.5, op0=Alu.mult, op1=Alu.add)
                    u8.append(u8t)
                st[b]["u8"] = u8
                st[b]["m1"] = {}
                st[b]["qy"] = {}
                st[b]["pscc"] = {}
                st[b]["qc"] = {0: [None, None], 1: [None, None]}
                st[b]["m3y"] = {}
                st[b]["m3c"] = {0: {}, 1: {}}

            def chunk_p1(b, jc, ab):
                # m1 tile layout: [128 col, 4 r, 32 grp, 8 kv]
                #   grp 0:16 = Y (g16), 16:24 = Cb (vb8), 24:32 = Cr (vb8)
                u8 = st[b]["u8"]
                if ab == 0:
                    st[b]["m1"][jc] = m1pool.tile([128, 4, 32, 8], DT16,
                                                  tag="m1", name="m1t")
                mt = st[b]["m1"][jc]
                ps = pspool.tile([128, 2, 256], DT, tag="ps", name="psp1")
                for g in range(2):
                    r = 2 * ab + g
                    for ch in range(3):
                        nc.tensor.matmul(
                            ps[:, g, :],
                            u8[r][:, 128 * jc:128 * (jc + 1), ch],
                            ct[f"mvp1_{ch}"],
                            start=(ch == 0), stop=False)
                    nc.tensor.matmul(
                        ps[:, g, :], ct["c1024"], ct["mvcomp"],
                        start=False, stop=True)
                if ab == 0:
                    nc.scalar.copy(mt[:, 0:2], ps[:])
                else:
                    nc.vector.tensor_copy(mt[:, 2:4], ps[:])

            def chunk_p2y(b, jc):
                m1 = st[b]["m1"][jc]
                ps2 = pspool.tile([128, 4, 16, 8], DT, tag="ps", name="ps2")
                for kv in range(8):
                    nc.tensor.matmul(
                        ps2[:, :, :, kv], ct[f"sp2y_{kv}"],
                        m1[:, :, 0:16, kv],
                        start=(kv == 0), stop=(kv == 7))
                qt = qpool.tile([128, 4, 16, 8], DT16, tag="qy", name="qty")
                nc.vector.tensor_scalar(
                    out=qt[:], in0=ps2[:], scalar1=C_RNE, scalar2=C_RNE,
                    op0=Alu.add, op1=Alu.subtract)
                nc.vector.tensor_tensor(
                    out=qt[:], in0=qt[:], in1=ct["taby"], op=Alu.mult)
                st[b]["qy"][jc] = qt

            def chunk_p2c(b, jc):
                m1 = st[b]["m1"][jc]
                psc = pspool.tile([128, 2, 4, 8, 8], DT, tag="ps", name="psc")
                idx = 0
                for chi in range(2):
                    for kv in range(8):
                        nc.tensor.matmul(
                            psc[:, chi, :, :, kv], ct[f"sp2c_{kv}"],
                            m1[:, :, 16 + 8 * chi:24 + 8 * chi, kv],
                            start=(idx == 0), stop=(idx == 15))
                        idx += 1
                st[b]["pscc"][jc] = psc

            def chunk_qc(b, t_):
                for chi in range(2):
                    qt = qpool.tile([128, 4, 8, 8], DT16, tag="qc", name="qtc")
                    for jcp in range(2):
                        psc = st[b]["pscc"][2 * t_ + jcp]
                        nc.vector.tensor_scalar(
                            out=qt[64 * jcp:64 * (jcp + 1)],
                            in0=psc[0:64, chi], scalar1=C_RNE,
                            scalar2=C_RNE, op0=Alu.add, op1=Alu.subtract)
                    nc.vector.tensor_tensor(
                        out=qt[:], in0=qt[:], in1=ct["tabc"], op=Alu.mult)
                    st[b]["qc"][chi][t_] = qt

            def chunk_p3y(b, r):
                qy = st[b]["qy"]
                ps3 = pspool.tile([128, 4, 128], DT, tag="ps", name="ps3")
                for jc in range(4):
                    nc.tensor.matmul(
                        ps3[:, jc, :], qy[jc][:, r], ct["mvp3y"],
                        start=(jc == 0), stop=(jc == 3))
                mt = m3pool.tile([128, 4, 128], DT16, tag="m3y", name="mty")
                nc.scalar.copy(mt[:], ps3[:])
                st[b]["m3y"][r] = mt

            def chunk_p3c(b, chi, h):
                qc = st[b]["qc"]
                ps3 = pspool.tile([128, 2, 256], DT, tag="ps", name="ps3c")
                for t_ in range(2):
                    nc.tensor.matmul(
                        ps3[:, t_, :], qc[chi][t_][:, 2 * h:2 * h + 2],
                        ct["mvp3c"],
                        start=(t_ == 0), stop=(t_ == 1))
                mt = m3pool.tile([128, 2, 256], DT16, tag="m3c", name="mtc")
                nc.scalar.copy(mt[:], ps3[:])
                st[b]["m3c"][chi][h] = mt

            def chunk_p4(b, r):
                rc, half = divmod(r, 2)
                psR = pspool.tile([128, 512], DT, tag="ps", name="psR")
                psG = pspool.tile([128, 512], DT, tag="ps", name="psG")
                psB4 = pspool.tile([128, 512], DT, tag="ps", name="psB4")
                my = st[b]["m3y"][r]
                mcb = st[b]["m3c"][0][rc]
                mcr = st[b]["m3c"][1][rc]
                nc.tensor.matmul(psR[:], ct["sp4y"], my[:],
                                 start=True, stop=False)
                nc.tensor.matmul(psG[:], ct["sp4y"], my[:],
                                 start=True, stop=False)
                nc.tensor.matmul(psB4[:], ct["sp4y"], my[:],
                                 start=True, stop=False)
                nc.tensor.matmul(psR[:], ct[f"sp4c_h{half}_rcr"],
                                 mcr[:], start=False, stop=True)
                nc.tensor.matmul(psG[:], ct[f"sp4c_h{half}_gcb"],
                                 mcb[:], start=False, stop=False)
                nc.tensor.matmul(psG[:], ct[f"sp4c_h{half}_gcr"],
                                 mcr[:], start=False, stop=True)
                nc.tensor.matmul(psB4[:], ct[f"sp4c_h{half}_bcb"],
                                 mcb[:], start=False, stop=True)
                ot = otpool.tile([128, 512, 3], mybir.dt.uint8, tag="o",
                                 name="ot")
                last = (b == B_PER_CORE - 1 and r >= 2)
                for chn, ps in ((0, psR), (2, psB4)):
                    nc.vector.tensor_scalar(
                        out=ot[:, :, chn], in0=ps[:], scalar1=255.0,
                        scalar2=0.0, op0=Alu.min, op1=Alu.max)
                if last:
                    pt = ppool.tile([128, 512], DT16, tag="pt", name="pt")
                    nc.scalar.copy(pt[:], psG[:])
                    nc.gpsimd.tensor_scalar(
                        out=ot[:, :, 1], in0=pt[:], scalar1=255.0,
                        scalar2=0.0, op0=Alu.min, op1=Alu.max)
                    nc.sync.dma_start(out=o_d[b, 128 * r:128 * (r + 1)],
                                      in_=ot[:])
                else:
                    pt = ppool.tile([128, 512], DT16, tag="pt", name="pt")
                    nc.scalar.copy(pt[:], psG[:])
                    nc.gpsimd.tensor_scalar(
                        out=ot[:, :, 1], in0=pt[:], scalar1=255.0,
                        scalar2=0.0, op0=Alu.min, op1=Alu.max)
                    nc.gpsimd.dma_start(out=o_d[b, 128 * r:128 * (r + 1)],
                                        in_=ot[:])

            def load_consts_rest():
                c1 = CONST_OFFS["c1024"][0] + CONST_OFFS["c1024"][1]
                c2 = CONST_OFFS["mvp3y"][0]
                tb = CONST_OFFS["taby"][0]
                nc.sync.dma_start(out=cbig[:, c1:c2], in_=cpak_d[:, c1:c2])
                nc.sync.dma_start(out=cbig[:, tb:], in_=cpak_d[:, tb:])
                nc.sync.dma_start(out=cbig[:, c2:tb], in_=cpak_d[:, c2:tb])

            def step(b, p):
                # A-chunks for image b interleaved with B-chunks for image p
                has_a = b is not None
                has_p = p is not None
                if b == 0:
                    c1 = CONST_OFFS["c1024"][0] + CONST_OFFS["c1024"][1]
                    nc.sync.dma_start(out=cbig[:, 0:c1], in_=cpak_d[:, 0:c1])
                    chunk_load_half(0, 0)
                    chunk_load_half(0, 1)
                    chunk_load_half(1, 0)
                    load_consts_rest()
                if has_a:
                    for jc in range(4):
                        chunk_p1(b, jc, 0)
                if has_p:
                    chunk_p3c(p, 0, 0)
                    chunk_p3c(p, 1, 0)
                    chunk_p3y(p, 0)
                    chunk_p3y(p, 1)
                if b == 0:
                    chunk_load_half(1, 1)
                if has_a:
                    for jc in range(4):
                        chunk_p1(b, jc, 1)
                if has_p:
                    chunk_p4(p, 0)
                    chunk_p4(p, 1)
                if has_a and b + 1 < B_PER_CORE and b >= 1:
                    chunk_load_half(b + 1, 0)
                if has_a:
                    chunk_p2y(b, 0)
                    chunk_p2c(b, 0)
                    chunk_p2y(b, 1)
                    chunk_p2c(b, 1)
                    chunk_qc(b, 0)
                if has_p:
                    chunk_p3c(p, 0, 1)
                    chunk_p3c(p, 1, 1)
                    chunk_p3y(p, 2)
                    chunk_p3y(p, 3)
                if has_a and b + 1 < B_PER_CORE and b >= 1:
                    chunk_load_half(b + 1, 1)
                if has_a:
                    chunk_p2y(b, 2)
                    chunk_p2c(b, 2)
                    chunk_p2y(b, 3)
                    chunk_p2c(b, 3)
                    chunk_qc(b, 1)
                if has_p:
                    chunk_p4(p, 2)
                    chunk_p4(p, 3)

            for s in range(B_PER_CORE + 1):
                step(s if s < B_PER_CORE else None,
                     s - 1 if s >= 1 else None)

    nc.compile()
    return nc


_CACHE = {}


def kernel(x: np.ndarray) -> np.ndarray:
    assert x.shape == (32, 512, 512, 3)
    if "nc" not in _CACHE:
        _CACHE["nc"] = build_nc()
        _CACHE["consts"] = build_consts()
    nc = _CACHE["nc"]
    consts = _CACHE["consts"]
    xs = np.ascontiguousarray(x.astype(F))
    in_maps = []
    for i in range(N_CORES):
        m = {"x": xs[B_PER_CORE * i:B_PER_CORE * (i + 1)]}
        m.update(consts)
        in_maps.append(m)
    res = run_bass_kernel_spmd(nc, in_maps, list(range(N_CORES)))
    out = np.concatenate([res.results[i]["out"] for i in range(N_CORES)], axis=0)
    return out.astype(np.float32) / np.float32(255.0)
